# revision 21
# baseline (speedup 1.0000x reference)
"""Trainium2 Bass kernel for NeuralPCG GNN message passing (8 NeuronCores).

Strategy: destination-sharded edges (core k owns all edges whose dest node is
in its 2500-node range), feature-major fp16 matmuls, dma_gather for P/Q lookups,
one-hot SEL matmuls for segment sums, one AllGather of the fused P|Q table per
message-passing step.
"""
import os
import numpy as np
import ml_dtypes
from contextlib import ExitStack

import concourse.bass as bass
import concourse.tile as tile
from concourse import bacc, mybir
from concourse.bass_utils import run_bass_kernel_spmd

N = 20000
E = 320000
L = 128
S = 3
NCORES = 8
NB = 2500            # nodes per core
BLOCKS = 20          # 128-node blocks per core
NPAD = BLOCKS * 128  # 2560
TROWS = NCORES * NPAD  # 20480 rows in the AllGathered PQ table
GT = 6               # tiles per gather group (6*128 = 768 idxs;
                     # dma_gather with num_idxs=1024 hangs the device)

F32 = mybir.dt.float32
F16 = mybir.dt.float16
F8 = mybir.dt.float8e4
I16 = mybir.dt.int16
AF = mybir.ActivationFunctionType
OP = mybir.AluOpType

NP16 = np.float16
NP8 = ml_dtypes.float8_e4m3fn

_CACHE = {}


# ----------------------------------------------------------------------------
# Host-side graph preprocessing (index manipulation + sharding only)
# ----------------------------------------------------------------------------

def _wrap_idxs(idx):
    """[n] int -> [128, n//16] int16 wrapped layout for dma_gather."""
    n = idx.shape[0]
    assert n % 16 == 0
    block = idx.reshape(n // 16, 16).T.astype(np.int16)
    return np.tile(block, (8, 1))


def _prep(x, edge_attr, edge_index):
    row = np.asarray(edge_index[0]).astype(np.int64)
    col = np.asarray(edge_index[1]).astype(np.int64)
    ea = np.asarray(edge_attr).reshape(-1).astype(np.float32)
    xf = np.asarray(x).reshape(-1).astype(np.float32)

    cnt_full = np.bincount(row, minlength=N).astype(np.float32)
    core_of = row // NB

    cores = []
    ebc_max = 0
    for k in range(NCORES):
        eids = np.nonzero(core_of == k)[0]
        order = np.argsort(row[eids], kind="stable")
        eids = eids[order]
        blk = (row[eids] - k * NB) // 128
        bc = np.bincount(blk, minlength=BLOCKS)
        ebc_max = max(ebc_max, int(bc.max()))
        cores.append((eids, blk, bc))

    Tb = max(2, (ebc_max + 127) // 128)
    EB = Tb * 128
    Epad = BLOCKS * EB
    ET = Epad // 128  # number of 128-edge tiles
    NCH = Epad // 512  # always integer: Epad = 20*Tb*128

    def trow(n):
        return (n // NB) * NPAD + (n % NB)

    per_core = []
    for k in range(NCORES):
        eids, blk, bc = cores[k]
        r = row[eids]
        c = col[eids]
        starts = np.zeros(BLOCKS, dtype=np.int64)
        np.cumsum(bc[:-1], out=starts[1:])
        pos_in_blk = np.arange(len(eids)) - starts[blk]
        dst = blk * EB + pos_in_blk

        gp = np.zeros(Epad, dtype=np.int64)
        gq = np.zeros(Epad, dtype=np.int64)
        slot = np.full(Epad, -1, dtype=np.int64)
        ea_s = np.ones(Epad, dtype=np.float32)
        dm = np.zeros(Epad, dtype=np.float32)
        orig = np.full(Epad, -1, dtype=np.int64)

        gp[dst] = trow(r)
        gq[dst] = trow(c)
        slot[dst] = (r - k * NB) % 128
        ea_s[dst] = ea[eids]
        dm[dst] = (r == c).astype(np.float32)
        orig[dst] = eids

        sel = np.zeros((Epad, 128), dtype=NP8)
        valid = slot >= 0
        sel[np.nonzero(valid)[0], slot[valid]] = NP8(1.0)
        selT = np.ascontiguousarray(sel.T)  # [128 slots, Epad]

        own = cnt_full[k * NB:(k + 1) * NB]
        tmp = np.zeros(BLOCKS * 128, dtype=np.float32)
        tmp[:NB] = own
        cnt_nm = tmp.reshape(BLOCKS, 128).T.copy()
        ind = np.zeros((1, NPAD), dtype=np.float32)
        ind[0, :NB] = (own > 0).astype(np.float32)

        x_own = np.zeros(NPAD, dtype=np.float32)
        x_own[:NB] = xf[k * NB:(k + 1) * NB]

        em = lambda a: a.reshape(ET, 128).T.copy()  # edge-slot-major [128, ET]
        per_core.append(dict(
            ea_r=ea_s.reshape(NCH, 512),
            ea_em=em(ea_s),
            dm_em=em(dm).astype(NP16),
            dmc_em=em((1.0 - dm) * (slot >= 0)).astype(NP16),
            gq_idx=_wrap_idxs(gq),
            sel=sel,
            selT=selT,
            x_r=x_own.reshape(NPAD // 512, 512),
            cnt_nm=cnt_nm,
            ind_r=ind,
            orig=orig,
        ))
    return per_core, Tb


def _weights_inputs(inp):
    """Build the weight/bias input arrays (shared across cores)."""
    g = lambda name: np.asarray(inp[name], dtype=np.float32)
    w = {}
    col = lambda a: a.reshape(128, 1).astype(np.float32)

    w["encn_W0"] = g("encn_W0").reshape(1, L)
    w["encn_b0"] = col(g("encn_b0"))
    w["encn_W1h"] = g("encn_W1").astype(NP16)
    w["encn_b1"] = col(g("encn_b1"))
    w["ence_W0"] = g("ence_W0").reshape(1, L)
    w["ence_b0"] = col(g("ence_b0"))
    w["ence_W1h"] = g("ence_W1").astype(NP16)
    w["ence_b1"] = col(g("ence_b1"))
    eW0, eb0, eW1, eb1 = g("eW0"), g("eb0"), g("eW1"), g("eb1")
    nW0, nb0, nW1, nb1 = g("nW0"), g("nb0"), g("nW1"), g("nb1")
    for s in range(S):
        w[f"eW0ab_{s}"] = np.concatenate(
            [eW0[s, :L, :], eW0[s, L:2 * L, :]], axis=1).astype(NP16)
        w[f"eW0c_{s}"] = eW0[s, 2 * L:, :].astype(NP16)
        w[f"eb0r_{s}"] = eb0[s].reshape(1, L).astype(NP16)
        w[f"eW1h_{s}"] = eW1[s].astype(NP16)
        w[f"eb1_{s}"] = col(eb1[s])
    for s in range(S - 1):
        w[f"eW1f_{s}"] = eW1[s]
        w[f"eb1row_{s}"] = eb1[s].reshape(1, L)
        w[f"nW0a_{s}"] = nW0[s, :L, :]
        w[f"nW0bh_{s}"] = nW0[s, L:, :].astype(NP16)
        w[f"nb0_{s}"] = col(nb0[s])
        w[f"nW1h_{s}"] = nW1[s].astype(NP16)
        w[f"nb1_{s}"] = col(nb1[s])
    w["dec_W0h"] = g("dec_W0").astype(NP16)
    w["dec_b0"] = col(g("dec_b0"))
    w["dec_W1h"] = g("dec_W1").reshape(L, 1).astype(NP16)
    w["dec_b1"] = np.full((128, 1), float(np.asarray(inp["dec_b1"]).reshape(-1)[0]),
                          dtype=np.float32)
    w["ident"] = np.eye(128, dtype=np.float32)
    w["ident16"] = np.eye(128, dtype=NP16)
    w["ones512"] = np.ones((1, 512), dtype=NP16)
    return w


# ----------------------------------------------------------------------------
# Device program
# ----------------------------------------------------------------------------

def _build(nc, Tb, w_shapes):
    kb_blocks = int(os.environ.get("KB_BLOCKS", str(BLOCKS)))
    kb_no_sel = bool(int(os.environ.get("KB_NO_SEL", "0")))
    kb_no_tp = bool(int(os.environ.get("KB_NO_TP", "0")))
    kb_no_gather = bool(int(os.environ.get("KB_NO_GATHER", "0")))
    kb_no_cc = bool(int(os.environ.get("KB_NO_CC", "0")))
    kb_no_dec = bool(int(os.environ.get("KB_NO_DEC", "0")))
    EB = Tb * 128
    Epad = BLOCKS * EB
    ET = Epad // 128
    NCH = Epad // 512
    # gather groups (in tiles) per block
    groups = []
    t0 = 0
    while t0 < Tb:
        groups.append((t0, min(GT, Tb - t0)))
        t0 += GT

    din = {}

    def inp(name, shape, dtype):
        din[name] = nc.dram_tensor(name, shape, dtype, kind="ExternalInput")
        return din[name]

    inp("ea_r", [NCH, 512], F32)
    inp("ea_em", [128, ET], F32)
    inp("dm_em", [128, ET], F16)
    inp("dmc_em", [128, ET], F16)
    inp("gq_idx", [128, Epad // 16], I16)
    inp("sel", [Epad, 128], F8)
    inp("selT", [128, Epad], F8)
    inp("x_r", [NPAD // 512, 512], F32)
    inp("cnt_nm", [128, BLOCKS], F32)
    inp("ind_r", [1, NPAD], F32)
    for name, arr_shape, np_dtype in w_shapes:
        inp(name, list(arr_shape), F16 if np_dtype == NP16 else F32)

    out_em = nc.dram_tensor("out_em", [128, ET], F32, kind="ExternalOutput")

    with tile.TileContext(nc) as tc, ExitStack() as ctx:
        P = lambda name, bufs, **kw: ctx.enter_context(
            tc.tile_pool(name=name, bufs=bufs, **kw))
        const = P("const", 1)
        big = P("big", 1)
        dram = P("dram", 1, space="DRAM")
        selp = P("selp", 3)
        selTring = P("selTring", 3)
        gath = P("gath", 4)
        hring = P("hring", 6)
        hem = P("hem", 8)
        tring = P("tring", 4)
        sgring = P("sgring", 3)
        rows = P("rows", 2)
        aggring = P("aggring", 2)
        pqring = P("pqring", 2)
        ps_r = P("ps_r", 2, space="PSUM")
        ps_le = P("ps_le", 2, space="PSUM")
        ps_g = P("ps_g", 1, space="PSUM")
        ps_m = P("ps_m", 1, space="PSUM")
        ps_t = P("ps_t", 2, space="PSUM")

        # ---- load constants / weights ----
        W = {}
        for name, arr_shape, np_dtype in w_shapes:
            t = const.tile(list(arr_shape), F16 if np_dtype == NP16 else F32,
                           name=f"w_{name}")
            nc.sync.dma_start(t[:], din[name][:])
            W[name] = t
        gq_idx = const.tile([128, Epad // 16], I16, name="gq_idx_s")
        nc.sync.dma_start(gq_idx[:], din["gq_idx"][:])
        cnt = const.tile([128, BLOCKS], F32, name="cnt_s")
        nc.sync.dma_start(cnt[:], din["cnt_nm"][:])
        ind = const.tile([1, NPAD], F32, name="ind_s")
        nc.sync.dma_start(ind[:], din["ind_r"][:])
        ea_em = const.tile([128, ET], F32, name="ea_em_s")
        nc.sync.dma_start(ea_em[:], din["ea_em"][:])
        dm_em = const.tile([128, ET], F16, name="dm_em_s")
        nc.sync.dma_start(dm_em[:], din["dm_em"][:])
        dmc_em = const.tile([128, ET], F16, name="dmc_em_s")
        nc.sync.dma_start(dmc_em[:], din["dmc_em"][:])

        invc = const.tile([128, BLOCKS], F32, name="invc")
        nc.vector.tensor_scalar_max(invc[:], cnt[:], 1.0)
        nc.vector.reciprocal(invc[:], invc[:])

        # ---- persistent big tensors ----
        le = big.tile([128, Epad], F16, name="le")       # edge latent (feature-major)
        ln = big.tile([128, NPAD], F32, name="ln")       # own-node latent
        sst = big.tile([128, NPAD], F32, name="sst")     # scaled segsum(h)^T
        p_own = big.tile([128, NPAD], F16, name="p_own")  # [slot, f] per block
        dec_em = big.tile([128, ET], F32, name="dec_em")
        cmb = big.tile([128, ET], F32, name="cmb")

        if kb_no_sel or kb_blocks < BLOCKS:
            nc.vector.memset(sst[:], 0.0)
        q_own = [dram.tile([NPAD, 128], F16, name=f"q_own_{s}") for s in range(S)]
        q_full = [dram.tile([TROWS, 128], F16, name=f"q_full_{s}",
                            addr_space="Shared") for s in range(S)]

        def mlp_rows(src_dram, nrows, hidden_W0, b0, W1h, b1, dst):
            """dst[:, 512j:...] = W1h.T @ relu(W0 (x) row_j + b0) + b1."""
            for j in range(nrows):
                r = rows.tile([1, 512], F32, tag="rowin")
                nc.sync.dma_start(r[:], src_dram[j:j + 1, :])
                ps = ps_m.tile([128, 512], F32, tag="m")
                nc.tensor.matmul(ps[:], hidden_W0[:], r[:], start=True, stop=True)
                h0 = hring.tile([128, 512], F16, tag="h")
                nc.scalar.activation(h0[:], ps[:], AF.Relu, bias=b0[:])
                ps2 = ps_le.tile([128, 512], F32, tag="le")
                nc.tensor.matmul(ps2[:], W1h[:], h0[:], start=True, stop=True)
                nc.scalar.activation(dst[:, 512 * j:512 * (j + 1)], ps2[:],
                                     AF.Identity, bias=b1[:])

        # ---- encoders ----
        mlp_rows(din["x_r"], NPAD // 512, W["encn_W0"], W["encn_b0"],
                 W["encn_W1h"], W["encn_b1"], ln)
        mlp_rows(din["ea_r"], NCH, W["ence_W0"], W["ence_b0"],
                 W["ence_W1h"], W["ence_b1"], le)

        def pq_pass(s):
            """P half -> p_own (SBUF, [slot, f] per block); Q half -> AllGather."""
            for j in range(BLOCKS):
                l16 = hem.tile([128, 128], F16, tag="hem")
                nc.vector.tensor_copy(l16[:], ln[:, 128 * j:128 * (j + 1)])
                ps = ps_m.tile([128, 256], F32, tag="m")
                nc.tensor.matmul(ps[:], l16[:], W[f"eW0ab_{s}"][:],
                                 start=True, stop=True)
                nc.scalar.activation(p_own[:, 128 * j:128 * (j + 1)], ps[:, 0:128],
                                     AF.Copy)
                t = pqring.tile([128, 128], F16, tag="pqe")
                nc.vector.tensor_copy(t[:], ps[:, 128:256])
                nc.sync.dma_start(q_own[s][128 * j:128 * (j + 1), :], t[:])
            if kb_no_cc:
                nc.sync.dma_start(q_full[s][0:NPAD, :], q_own[s][:])
            else:
                nc.gpsimd.collective_compute(
                    "AllGather", OP.bypass,
                    replica_groups=[list(range(NCORES))],
                    ins=[q_own[s].opt()],
                    outs=[q_full[s].opt()],
                )

        pq_pass(0)

        g_ctr = 0
        for s in range(S):
            do_agg = s < S - 1
            eW0c = W[f"eW0c_{s}"]
            eW1h = W[f"eW1h_{s}"]
            eb0r = W[f"eb0r_{s}"]
            eb1 = W[f"eb1_{s}"]
            for b in range(kb_blocks):
                selT_b = selTring.tile([128, EB], F8, tag="selT")
                nc.sync.dma_start(selT_b[:], din["selT"][:, b * EB:(b + 1) * EB])
                pob = p_own[:, 128 * b:128 * (b + 1)]
                if do_agg and not kb_no_sel:
                    g_ps = ps_g.tile([128, 128], F32, tag="g")
                for (gt0, gnt) in groups:
                    # gather this group's Q rows
                    i0 = b * EB + gt0 * 128
                    ni = gnt * 128
                    gq_t = gath.tile([128, 1, GT * 128], F16, tag="gq")
                    if kb_no_gather:
                        nc.vector.memset(gq_t[:], 0.0)
                    else:
                        nc.gpsimd.dma_gather(
                            gq_t[:, :, :ni], q_full[s][:, :],
                            gq_idx[:, i0 // 16:(i0 + ni) // 16],
                            num_idxs=ni, num_idxs_reg=ni,
                            elem_size=128, elem_step=128, transpose=True,
                            queue_num=g_ctr % 4)
                        g_ctr += 1
                    if do_agg and not kb_no_sel:
                        sel_t = selp.tile([128, GT, 128], F8, tag="sel")
                        nc.sync.dma_start(
                            sel_t[:, :gnt, :],
                            din["sel"][i0:i0 + ni, :].rearrange(
                                "(t p) s -> p t s", p=128))
                    # chunks of <=512 within the group
                    co = 0
                    while co < ni:
                        cw = min(512, ni - co)
                        goff = i0 + co            # global edge-slot offset
                        boff = gt0 * 128 + co     # offset within the block
                        ps = ps_r.tile([128, 512], F32, tag="r")
                        nc.tensor.matmul(ps[:, :cw], pob,
                                         selT_b[:, boff:boff + cw],
                                         start=True, stop=False)
                        nc.tensor.matmul(ps[:, :cw], eb0r[:],
                                         W["ones512"][:, :cw],
                                         start=False, stop=False)
                        nc.tensor.matmul(ps[:, :cw], eW0c[:],
                                         le[:, goff:goff + cw],
                                         start=False, stop=True)
                        t2 = tring.tile([128, 512], F16, tag="t")
                        nc.vector.tensor_tensor(t2[:, :cw],
                                                gq_t[:, 0, co:co + cw],
                                                ps[:, :cw], op=OP.add)
                        h = hring.tile([128, 512], F16, tag="h")
                        nc.vector.tensor_scalar_max(h[:, :cw], t2[:, :cw], 0.0)
                        ps2 = ps_le.tile([128, 512], F32, tag="le")
                        nc.tensor.matmul(ps2[:, :cw], eW1h[:], h[:, :cw],
                                         start=True, stop=True)
                        nc.scalar.activation(le[:, goff:goff + cw], ps2[:, :cw],
                                             AF.Identity, bias=eb1[:])
                        if do_agg and not kb_no_sel:
                            for u in range(cw // 128):
                                tt = gt0 + (co // 128) + u
                                he = hem.tile([128, 128], F16, tag="hem")
                                if kb_no_tp:
                                    nc.vector.tensor_copy(
                                        he[:], h[:, 128 * u:128 * (u + 1)])
                                else:
                                    ht_ps = ps_t.tile([128, 128], F16, tag="tp")
                                    nc.tensor.transpose(
                                        ht_ps[:], h[:, 128 * u:128 * (u + 1)],
                                        W["ident16"][:])
                                    if tt % 2 == 0:
                                        nc.scalar.activation(he[:], ht_ps[:],
                                                             AF.Copy)
                                    else:
                                        nc.vector.tensor_copy(he[:], ht_ps[:])
                                nc.tensor.matmul(
                                    g_ps[:], sel_t[:, (co // 128) + u, :], he[:],
                                    start=(tt == 0), stop=(tt == Tb - 1),
                                    skip_group_check=True)
                        co += cw
                if do_agg and not kb_no_sel:
                    sg = sgring.tile([128, 128], F32, tag="sg")
                    nc.vector.tensor_scalar_mul(sg[:], g_ps[:], invc[:, b:b + 1])
                    ps_tr = ps_m.tile([128, 128], F32, tag="m")
                    nc.tensor.transpose(ps_tr[:], sg[:], W["ident"][:])
                    nc.vector.tensor_copy(sst[:, 128 * b:128 * (b + 1)], ps_tr[:])

            if do_agg:
                # agg + node MLP, then next-step PQ + AllGather
                for j in range(NPAD // 512):
                    o = 512 * j
                    a_ps = ps_m.tile([128, 512], F32, tag="m")
                    nc.tensor.matmul(a_ps[:], W[f"eW1f_{s}"][:], sst[:, o:o + 512],
                                     start=True, stop=False)
                    nc.tensor.matmul(a_ps[:], W[f"eb1row_{s}"][:],
                                     ind[:, o:o + 512], start=False, stop=True)
                    agg = aggring.tile([128, 512], F16, tag="agg16")
                    nc.scalar.activation(agg[:], a_ps[:], AF.Copy)
                    p_ps = ps_r.tile([128, 512], F32, tag="r")
                    nc.tensor.matmul(p_ps[:], W[f"nW0a_{s}"][:], ln[:, o:o + 512],
                                     start=True, stop=False)
                    nc.tensor.matmul(p_ps[:], W[f"nW0bh_{s}"][:], agg[:],
                                     start=False, stop=True)
                    hn = hring.tile([128, 512], F16, tag="h")
                    nc.scalar.activation(hn[:], p_ps[:], AF.Relu,
                                         bias=W[f"nb0_{s}"][:])
                    l_ps = ps_le.tile([128, 512], F32, tag="le")
                    nc.tensor.matmul(l_ps[:], W[f"nW1h_{s}"][:], hn[:],
                                     start=True, stop=True)
                    nc.scalar.activation(ln[:, o:o + 512], l_ps[:],
                                         AF.Identity, bias=W[f"nb1_{s}"][:])
                pq_pass(s + 1)

        # ---- decoder ----
        for ci in range(0 if kb_no_dec else NCH):
            off = 512 * ci
            ps = ps_r.tile([128, 512], F32, tag="r")
            nc.tensor.matmul(ps[:], W["dec_W0h"][:], le[:, off:off + 512],
                             start=True, stop=True)
            hd = hring.tile([128, 512], F16, tag="h")
            nc.scalar.activation(hd[:], ps[:], AF.Relu, bias=W["dec_b0"][:])
            d_ps = ps_m.tile([128, 4], F32, tag="m")
            for u in range(4):
                nc.tensor.matmul(d_ps[:, u:u + 1], hd[:, 128 * u:128 * (u + 1)],
                                 W["dec_W1h"][:], start=True, stop=True)
            nc.vector.tensor_scalar_add(dec_em[:, 4 * ci:4 * ci + 4], d_ps[:],
                                        W["dec_b1"][:])

        # ---- final combine: out = dm*0.5*sqrt(ea) + dmc*dec ----
        nc.scalar.sqrt(cmb[:], ea_em[:])
        nc.vector.scalar_tensor_tensor(cmb[:], dm_em[:], 0.5, cmb[:],
                                       op0=OP.mult, op1=OP.mult)
        nc.vector.tensor_tensor(dec_em[:], dmc_em[:], dec_em[:], op=OP.mult)
        nc.vector.tensor_tensor(cmb[:], cmb[:], dec_em[:], op=OP.add)
        nc.sync.dma_start(out_em[:], cmb[:])

    nc.compile()


# ----------------------------------------------------------------------------
# Entry point
# ----------------------------------------------------------------------------

def _get_program(Tb, w_shapes):
    key = Tb
    if key not in _CACHE:
        import time
        t0 = time.time()
        nc = bacc.Bacc("TRN2", target_bir_lowering=False, debug=False,
                       num_devices=NCORES, num_swdge_queues=4)
        _build(nc, Tb, w_shapes)
        if os.environ.get("KERNEL_VERBOSE"):
            print(f"[kernel] build+schedule+compile: {time.time()-t0:.1f}s",
                  flush=True)
        _CACHE[key] = nc
    return _CACHE[key]


def kernel(**inputs):
    per_core, Tb = _prep(inputs["x"], inputs["edge_attr"], inputs["edge_index"])
    w = _weights_inputs(inputs)
    w_shapes = [(k, v.shape, v.dtype.type) for k, v in w.items()]
    nc = _get_program(Tb, w_shapes)

    in_maps = []
    for k in range(NCORES):
        m = dict(w)
        pc = per_core[k]
        for key in ("ea_r", "ea_em", "dm_em", "dmc_em", "gq_idx",
                    "sel", "selT", "x_r", "cnt_nm", "ind_r"):
            m[key] = pc[key]
        in_maps.append(m)

    trace = bool(int(os.environ.get("KERNEL_TRACE", "0")))
    import time as _time
    _t0 = _time.time()
    res = run_bass_kernel_spmd(
        nc, in_maps, core_ids=list(range(NCORES)), trace=trace,
        tmpdir=os.environ.get("KERNEL_TRACE_DIR") or None)
    if os.environ.get("KERNEL_VERBOSE"):
        print(f"[kernel] exec phase: {_time.time()-_t0:.1f}s", flush=True)
    if trace:
        print(f"HW exec time: {res.exec_time_ns} ns")
        if res.instructions_and_trace:
            print("trace:", res.instructions_and_trace[1])

    out = np.zeros((E, 1), dtype=np.float32)
    ET = (BLOCKS * Tb * 128) // 128
    for k in range(NCORES):
        o = res.results[k]["out_em"]           # [128, ET]
        flat = o.T.reshape(-1)                 # slot order
        orig = per_core[k]["orig"]
        valid = orig >= 0
        out[orig[valid], 0] = flat[valid]
    return out



# revision 23
# speedup vs baseline: 1.3793x; 1.3793x over previous
"""Trainium2 Bass kernel for NeuralPCG GNN message passing (8 NeuronCores).

Strategy: destination-sharded edges (core k owns all edges whose dest node is
in its 2500-node range), feature-major fp16 matmuls, dma_gather for P/Q lookups,
one-hot SEL matmuls for segment sums, one AllGather of the fused P|Q table per
message-passing step.
"""
import os
import numpy as np
import ml_dtypes
from contextlib import ExitStack

import concourse.bass as bass
import concourse.tile as tile
from concourse import bacc, mybir
from concourse.bass_utils import run_bass_kernel_spmd

N = 20000
E = 320000
L = 128
S = 3
NCORES = 8
NB = 2500            # nodes per core
BLOCKS = 20          # 128-node blocks per core
NPAD = BLOCKS * 128  # 2560
TROWS = NCORES * NPAD  # 20480 rows in the AllGathered PQ table
GT = 6               # tiles per gather group (6*128 = 768 idxs;
                     # dma_gather with num_idxs=1024 hangs the device)

F32 = mybir.dt.float32
F16 = mybir.dt.float16
F8 = mybir.dt.float8e4
I16 = mybir.dt.int16
AF = mybir.ActivationFunctionType
OP = mybir.AluOpType

NP16 = np.float16
NP8 = ml_dtypes.float8_e4m3fn

_CACHE = {}


# ----------------------------------------------------------------------------
# Host-side graph preprocessing (index manipulation + sharding only)
# ----------------------------------------------------------------------------

def _wrap_idxs(idx):
    """[n] int -> [128, n//16] int16 wrapped layout for dma_gather."""
    n = idx.shape[0]
    assert n % 16 == 0
    block = idx.reshape(n // 16, 16).T.astype(np.int16)
    return np.tile(block, (8, 1))


def _prep(x, edge_attr, edge_index):
    row = np.asarray(edge_index[0]).astype(np.int64)
    col = np.asarray(edge_index[1]).astype(np.int64)
    ea = np.asarray(edge_attr).reshape(-1).astype(np.float32)
    xf = np.asarray(x).reshape(-1).astype(np.float32)

    cnt_full = np.bincount(row, minlength=N).astype(np.float32)
    core_of = row // NB

    cores = []
    ebc_max = 0
    for k in range(NCORES):
        eids = np.nonzero(core_of == k)[0]
        order = np.argsort(row[eids], kind="stable")
        eids = eids[order]
        blk = (row[eids] - k * NB) // 128
        bc = np.bincount(blk, minlength=BLOCKS)
        ebc_max = max(ebc_max, int(bc.max()))
        cores.append((eids, blk, bc))

    Tb = max(2, (ebc_max + 127) // 128)
    EB = Tb * 128
    Epad = BLOCKS * EB
    ET = Epad // 128  # number of 128-edge tiles
    NCH = Epad // 512  # always integer: Epad = 20*Tb*128

    def trow(n):
        return (n // NB) * NPAD + (n % NB)

    per_core = []
    for k in range(NCORES):
        eids, blk, bc = cores[k]
        r = row[eids]
        c = col[eids]
        starts = np.zeros(BLOCKS, dtype=np.int64)
        np.cumsum(bc[:-1], out=starts[1:])
        pos_in_blk = np.arange(len(eids)) - starts[blk]
        dst = blk * EB + pos_in_blk

        gp = np.zeros(Epad, dtype=np.int64)
        gq = np.zeros(Epad, dtype=np.int64)
        slot = np.full(Epad, -1, dtype=np.int64)
        ea_s = np.ones(Epad, dtype=np.float32)
        dm = np.zeros(Epad, dtype=np.float32)
        orig = np.full(Epad, -1, dtype=np.int64)

        gp[dst] = trow(r)
        gq[dst] = trow(c)
        slot[dst] = (r - k * NB) % 128
        ea_s[dst] = ea[eids]
        dm[dst] = (r == c).astype(np.float32)
        orig[dst] = eids

        sel = np.zeros((Epad, 128), dtype=NP8)
        valid = slot >= 0
        sel[np.nonzero(valid)[0], slot[valid]] = NP8(1.0)
        selT = np.ascontiguousarray(sel.T)  # [128 slots, Epad]

        own = cnt_full[k * NB:(k + 1) * NB]
        tmp = np.zeros(BLOCKS * 128, dtype=np.float32)
        tmp[:NB] = own
        cnt_nm = tmp.reshape(BLOCKS, 128).T.copy()
        ind = np.zeros((1, NPAD), dtype=np.float32)
        ind[0, :NB] = (own > 0).astype(np.float32)

        x_own = np.zeros(NPAD, dtype=np.float32)
        x_own[:NB] = xf[k * NB:(k + 1) * NB]

        em = lambda a: a.reshape(ET, 128).T.copy()  # edge-slot-major [128, ET]
        per_core.append(dict(
            ea_r=ea_s.reshape(NCH, 512),
            ea_em=em(ea_s),
            dm_em=em(dm).astype(NP16),
            dmc_em=em((1.0 - dm) * (slot >= 0)).astype(NP16),
            gq_idx=_wrap_idxs(gq),
            sel=sel,
            selT=selT,
            x_r=x_own.reshape(NPAD // 512, 512),
            cnt_nm=cnt_nm,
            ind_r=ind,
            orig=orig,
        ))
    return per_core, Tb


def _weights_inputs(inp):
    """Build the weight/bias input arrays (shared across cores)."""
    g = lambda name: np.asarray(inp[name], dtype=np.float32)
    w = {}
    col = lambda a: a.reshape(128, 1).astype(np.float32)

    w["encn_W0"] = g("encn_W0").reshape(1, L)
    w["encn_b0"] = col(g("encn_b0"))
    w["encn_W1h"] = g("encn_W1").astype(NP16)
    w["encn_b1"] = col(g("encn_b1"))
    w["ence_W0"] = g("ence_W0").reshape(1, L)
    w["ence_b0"] = col(g("ence_b0"))
    w["ence_W1h"] = g("ence_W1").astype(NP16)
    w["ence_b1"] = col(g("ence_b1"))
    eW0, eb0, eW1, eb1 = g("eW0"), g("eb0"), g("eW1"), g("eb1")
    nW0, nb0, nW1, nb1 = g("nW0"), g("nb0"), g("nW1"), g("nb1")
    for s in range(S):
        w[f"eW0ab_{s}"] = np.concatenate(
            [eW0[s, :L, :], eW0[s, L:2 * L, :]], axis=1).astype(NP16)
        w[f"eW0c_{s}"] = eW0[s, 2 * L:, :].astype(NP16)
        w[f"eb0bc_{s}"] = np.tile(eb0[s].reshape(1, L), (128, 1)).astype(NP16)
        w[f"eW1h_{s}"] = eW1[s].astype(NP16)
        w[f"eb1_{s}"] = col(eb1[s])
    for s in range(S - 1):
        w[f"eW1f_{s}"] = eW1[s]
        w[f"eb1row_{s}"] = eb1[s].reshape(1, L)
        w[f"nW0a_{s}"] = nW0[s, :L, :]
        w[f"nW0bh_{s}"] = nW0[s, L:, :].astype(NP16)
        w[f"nb0_{s}"] = col(nb0[s])
        w[f"nW1h_{s}"] = nW1[s].astype(NP16)
        w[f"nb1_{s}"] = col(nb1[s])
    w["dec_W0h"] = g("dec_W0").astype(NP16)
    w["dec_b0"] = col(g("dec_b0"))
    w["dec_W1h"] = g("dec_W1").reshape(L, 1).astype(NP16)
    w["dec_b1"] = np.full((128, 1), float(np.asarray(inp["dec_b1"]).reshape(-1)[0]),
                          dtype=np.float32)
    w["ident"] = np.eye(128, dtype=np.float32)
    w["ident16"] = np.eye(128, dtype=NP16)
    return w


# ----------------------------------------------------------------------------
# Device program
# ----------------------------------------------------------------------------

def _build(nc, Tb, w_shapes):
    kb_blocks = int(os.environ.get("KB_BLOCKS", str(BLOCKS)))
    kb_no_sel = bool(int(os.environ.get("KB_NO_SEL", "0")))
    kb_no_tp = bool(int(os.environ.get("KB_NO_TP", "0")))
    kb_no_gather = bool(int(os.environ.get("KB_NO_GATHER", "0")))
    kb_no_cc = bool(int(os.environ.get("KB_NO_CC", "0")))
    kb_no_dec = bool(int(os.environ.get("KB_NO_DEC", "0")))
    EB = Tb * 128
    Epad = BLOCKS * EB
    ET = Epad // 128
    NCH = Epad // 512
    # gather groups (in tiles) per block
    groups = []
    t0 = 0
    while t0 < Tb:
        groups.append((t0, min(GT, Tb - t0)))
        t0 += GT

    din = {}

    def inp(name, shape, dtype):
        din[name] = nc.dram_tensor(name, shape, dtype, kind="ExternalInput")
        return din[name]

    inp("ea_r", [NCH, 512], F32)
    inp("ea_em", [128, ET], F32)
    inp("dm_em", [128, ET], F16)
    inp("dmc_em", [128, ET], F16)
    inp("gq_idx", [128, Epad // 16], I16)
    inp("sel", [Epad, 128], F8)
    inp("selT", [128, Epad], F8)
    inp("x_r", [NPAD // 512, 512], F32)
    inp("cnt_nm", [128, BLOCKS], F32)
    inp("ind_r", [1, NPAD], F32)
    for name, arr_shape, np_dtype in w_shapes:
        inp(name, list(arr_shape), F16 if np_dtype == NP16 else F32)

    out_em = nc.dram_tensor("out_em", [128, ET], F32, kind="ExternalOutput")

    with tile.TileContext(nc) as tc, ExitStack() as ctx:
        P = lambda name, bufs, **kw: ctx.enter_context(
            tc.tile_pool(name=name, bufs=bufs, **kw))
        const = P("const", 1)
        big = P("big", 1)
        dram = P("dram", 1, space="DRAM")
        selp = P("selp", 3)
        selTring = P("selTring", 3)
        gath = P("gath", 4)
        hring = P("hring", 6)
        hem = P("hem", 8)
        tring = P("tring", 4)
        sgring = P("sgring", 3)
        rows = P("rows", 2)
        aggring = P("aggring", 2)
        pqring = P("pqring", 2)
        ps_r = P("ps_r", 2, space="PSUM")
        ps_le = P("ps_le", 2, space="PSUM")
        ps_g = P("ps_g", 1, space="PSUM")
        ps_m = P("ps_m", 1, space="PSUM")
        ps_t = P("ps_t", 2, space="PSUM")

        # ---- load constants / weights ----
        W = {}
        for name, arr_shape, np_dtype in w_shapes:
            t = const.tile(list(arr_shape), F16 if np_dtype == NP16 else F32,
                           name=f"w_{name}")
            nc.sync.dma_start(t[:], din[name][:])
            W[name] = t
        gq_idx = const.tile([128, Epad // 16], I16, name="gq_idx_s")
        nc.sync.dma_start(gq_idx[:], din["gq_idx"][:])
        cnt = const.tile([128, BLOCKS], F32, name="cnt_s")
        nc.sync.dma_start(cnt[:], din["cnt_nm"][:])
        ind = const.tile([1, NPAD], F32, name="ind_s")
        nc.sync.dma_start(ind[:], din["ind_r"][:])
        ea_em = const.tile([128, ET], F32, name="ea_em_s")
        nc.sync.dma_start(ea_em[:], din["ea_em"][:])
        dm_em = const.tile([128, ET], F16, name="dm_em_s")
        nc.sync.dma_start(dm_em[:], din["dm_em"][:])
        dmc_em = const.tile([128, ET], F16, name="dmc_em_s")
        nc.sync.dma_start(dmc_em[:], din["dmc_em"][:])

        zeros = const.tile([128, 512], F16, name="zeros")
        nc.vector.memset(zeros[:], 0.0)
        invc = const.tile([128, BLOCKS], F32, name="invc")
        nc.vector.tensor_scalar_max(invc[:], cnt[:], 1.0)
        nc.vector.reciprocal(invc[:], invc[:])

        # ---- persistent big tensors ----
        le = big.tile([128, Epad], F16, name="le")       # edge latent (feature-major)
        ln = big.tile([128, NPAD], F32, name="ln")       # own-node latent
        sst = big.tile([128, NPAD], F32, name="sst")     # scaled segsum(h)^T
        p_own = big.tile([128, NPAD], F16, name="p_own")  # [slot, f] per block
        dec_em = big.tile([128, ET], F32, name="dec_em")
        cmb = big.tile([128, ET], F32, name="cmb")

        if kb_no_sel or kb_blocks < BLOCKS:
            nc.vector.memset(sst[:], 0.0)
        q_own = [dram.tile([NPAD, 128], F16, name=f"q_own_{s}") for s in range(S)]
        q_full = [dram.tile([TROWS, 128], F16, name=f"q_full_{s}",
                            addr_space="Shared") for s in range(S)]

        def mlp_rows(src_dram, nrows, hidden_W0, b0, W1h, b1, dst):
            """dst[:, 512j:...] = W1h.T @ relu(W0 (x) row_j + b0) + b1."""
            for j in range(nrows):
                r = rows.tile([1, 512], F32, tag="rowin")
                nc.sync.dma_start(r[:], src_dram[j:j + 1, :])
                ps = ps_m.tile([128, 512], F32, tag="m")
                nc.tensor.matmul(ps[:], hidden_W0[:], r[:], start=True, stop=True)
                h0 = hring.tile([128, 512], F16, tag="h")
                nc.scalar.activation(h0[:], ps[:], AF.Relu, bias=b0[:])
                ps2 = ps_le.tile([128, 512], F32, tag="le")
                nc.tensor.matmul(ps2[:], W1h[:], h0[:], start=True, stop=True)
                nc.scalar.activation(dst[:, 512 * j:512 * (j + 1)], ps2[:],
                                     AF.Identity, bias=b1[:])

        # ---- encoders ----
        mlp_rows(din["x_r"], NPAD // 512, W["encn_W0"], W["encn_b0"],
                 W["encn_W1h"], W["encn_b1"], ln)
        mlp_rows(din["ea_r"], NCH, W["ence_W0"], W["ence_b0"],
                 W["ence_W1h"], W["ence_b1"], le)

        def pq_pass(s):
            """P half -> p_own (SBUF, [slot, f] per block); Q half -> AllGather."""
            for j in range(BLOCKS):
                l16 = hem.tile([128, 128], F16, tag="hem")
                nc.scalar.activation(l16[:], ln[:, 128 * j:128 * (j + 1)], AF.Copy)
                ps = ps_m.tile([128, 256], F32, tag="m")
                nc.tensor.matmul(ps[:], l16[:], W[f"eW0ab_{s}"][:],
                                 start=True, stop=True)
                nc.vector.tensor_tensor(
                    p_own[:, 128 * j:128 * (j + 1)], ps[:, 0:128],
                    W[f"eb0bc_{s}"][:], op=OP.add)
                t = pqring.tile([128, 128], F16, tag="pqe")
                nc.scalar.activation(t[:], ps[:, 128:256], AF.Copy)
                nc.sync.dma_start(q_own[s][128 * j:128 * (j + 1), :], t[:])
            if kb_no_cc:
                nc.sync.dma_start(q_full[s][0:NPAD, :], q_own[s][:])
            else:
                nc.gpsimd.collective_compute(
                    "AllGather", OP.bypass,
                    replica_groups=[list(range(NCORES))],
                    ins=[q_own[s].opt()],
                    outs=[q_full[s].opt()],
                )

        pq_pass(0)

        g_ctr = 0
        for s in range(S):
            do_agg = s < S - 1
            eW0c = W[f"eW0c_{s}"]
            eW1h = W[f"eW1h_{s}"]
            eb1 = W[f"eb1_{s}"]
            for b in range(kb_blocks):
                selT_b = selTring.tile([128, EB], F8, tag="selT")
                nc.sync.dma_start(selT_b[:], din["selT"][:, b * EB:(b + 1) * EB])
                pob = p_own[:, 128 * b:128 * (b + 1)]
                if do_agg and not kb_no_sel:
                    g_ps = ps_g.tile([128, 128], F32, tag="g")
                for (gt0, gnt) in groups:
                    # gather this group's Q rows
                    i0 = b * EB + gt0 * 128
                    ni = gnt * 128
                    gq_t = gath.tile([128, 1, GT * 128], F16, tag="gq")
                    if kb_no_gather:
                        nc.vector.memset(gq_t[:], 0.0)
                    else:
                        nc.gpsimd.dma_gather(
                            gq_t[:, :, :ni], q_full[s][:, :],
                            gq_idx[:, i0 // 16:(i0 + ni) // 16],
                            num_idxs=ni, num_idxs_reg=ni,
                            elem_size=128, elem_step=128, transpose=True,
                            queue_num=g_ctr % 4)
                        g_ctr += 1
                    if do_agg and not kb_no_sel:
                        sel_t = selp.tile([128, GT, 128], F8, tag="sel")
                        nc.sync.dma_start(
                            sel_t[:, :gnt, :],
                            din["sel"][i0:i0 + ni, :].rearrange(
                                "(t p) s -> p t s", p=128))
                    # chunks of <=512 within the group
                    co = 0
                    while co < ni:
                        cw = min(512, ni - co)
                        goff = i0 + co            # global edge-slot offset
                        boff = gt0 * 128 + co     # offset within the block
                        ps = ps_r.tile([128, 512], F32, tag="r")
                        nc.tensor.matmul(ps[:, :cw], pob,
                                         selT_b[:, boff:boff + cw],
                                         start=True, stop=False)
                        nc.tensor.matmul(ps[:, :cw], eW0c[:],
                                         le[:, goff:goff + cw],
                                         start=False, stop=True)
                        t2 = tring.tile([128, 512], F16, tag="t")
                        nc.vector.tensor_tensor(t2[:, :cw],
                                                gq_t[:, 0, co:co + cw],
                                                ps[:, :cw], op=OP.add)
                        h = hring.tile([128, 512], F16, tag="h")
                        nc.vector.tensor_tensor(h[:, :cw], t2[:, :cw],
                                                zeros[:, :cw], op=OP.max)
                        ps2 = ps_le.tile([128, 512], F32, tag="le")
                        nc.tensor.matmul(ps2[:, :cw], eW1h[:], h[:, :cw],
                                         start=True, stop=True)
                        nc.scalar.activation(le[:, goff:goff + cw], ps2[:, :cw],
                                             AF.Identity, bias=eb1[:])
                        if do_agg and not kb_no_sel:
                            for u in range(cw // 128):
                                tt = gt0 + (co // 128) + u
                                he = hem.tile([128, 128], F16, tag="hem")
                                if kb_no_tp:
                                    nc.vector.tensor_copy(
                                        he[:], h[:, 128 * u:128 * (u + 1)])
                                else:
                                    ht_ps = ps_t.tile([128, 128], F16, tag="tp")
                                    nc.tensor.transpose(
                                        ht_ps[:], h[:, 128 * u:128 * (u + 1)],
                                        W["ident16"][:])
                                    if tt % 2 == 0:
                                        nc.scalar.activation(he[:], ht_ps[:],
                                                             AF.Copy)
                                    else:
                                        nc.vector.tensor_tensor(
                                            he[:], ht_ps[:], zeros[:, 0:128],
                                            op=OP.add)
                                nc.tensor.matmul(
                                    g_ps[:], sel_t[:, (co // 128) + u, :], he[:],
                                    start=(tt == 0), stop=(tt == Tb - 1),
                                    skip_group_check=True)
                        co += cw
                if do_agg and not kb_no_sel:
                    sg = sgring.tile([128, 128], F32, tag="sg")
                    nc.vector.tensor_tensor(sg[:], g_ps[:],
                                            invc[:, b:b + 1].to_broadcast(
                                                [128, 128]), op=OP.mult)
                    ps_tr = ps_m.tile([128, 128], F32, tag="m")
                    nc.tensor.transpose(ps_tr[:], sg[:], W["ident"][:])
                    nc.vector.tensor_tensor(sst[:, 128 * b:128 * (b + 1)],
                                            ps_tr[:], zeros[:, 0:128], op=OP.add)

            if do_agg:
                # agg + node MLP, then next-step PQ + AllGather
                for j in range(NPAD // 512):
                    o = 512 * j
                    a_ps = ps_m.tile([128, 512], F32, tag="m")
                    nc.tensor.matmul(a_ps[:], W[f"eW1f_{s}"][:], sst[:, o:o + 512],
                                     start=True, stop=False)
                    nc.tensor.matmul(a_ps[:], W[f"eb1row_{s}"][:],
                                     ind[:, o:o + 512], start=False, stop=True)
                    agg = aggring.tile([128, 512], F16, tag="agg16")
                    nc.scalar.activation(agg[:], a_ps[:], AF.Copy)
                    p_ps = ps_r.tile([128, 512], F32, tag="r")
                    nc.tensor.matmul(p_ps[:], W[f"nW0a_{s}"][:], ln[:, o:o + 512],
                                     start=True, stop=False)
                    nc.tensor.matmul(p_ps[:], W[f"nW0bh_{s}"][:], agg[:],
                                     start=False, stop=True)
                    hn = hring.tile([128, 512], F16, tag="h")
                    nc.scalar.activation(hn[:], p_ps[:], AF.Relu,
                                         bias=W[f"nb0_{s}"][:])
                    l_ps = ps_le.tile([128, 512], F32, tag="le")
                    nc.tensor.matmul(l_ps[:], W[f"nW1h_{s}"][:], hn[:],
                                     start=True, stop=True)
                    nc.scalar.activation(ln[:, o:o + 512], l_ps[:],
                                         AF.Identity, bias=W[f"nb1_{s}"][:])
                pq_pass(s + 1)

        # ---- decoder ----
        for ci in range(0 if kb_no_dec else NCH):
            off = 512 * ci
            ps = ps_r.tile([128, 512], F32, tag="r")
            nc.tensor.matmul(ps[:], W["dec_W0h"][:], le[:, off:off + 512],
                             start=True, stop=True)
            hd = hring.tile([128, 512], F16, tag="h")
            nc.scalar.activation(hd[:], ps[:], AF.Relu, bias=W["dec_b0"][:])
            d_ps = ps_m.tile([128, 4], F32, tag="m")
            for u in range(4):
                nc.tensor.matmul(d_ps[:, u:u + 1], hd[:, 128 * u:128 * (u + 1)],
                                 W["dec_W1h"][:], start=True, stop=True)
            nc.vector.tensor_tensor(dec_em[:, 4 * ci:4 * ci + 4], d_ps[:],
                                    W["dec_b1"][:].to_broadcast([128, 4]),
                                    op=OP.add)

        # ---- final combine: out = dm*0.5*sqrt(ea) + dmc*dec ----
        nc.scalar.sqrt(cmb[:], ea_em[:])
        nc.vector.scalar_tensor_tensor(cmb[:], dm_em[:], 0.5, cmb[:],
                                       op0=OP.mult, op1=OP.mult)
        nc.vector.tensor_tensor(dec_em[:], dmc_em[:], dec_em[:], op=OP.mult)
        nc.vector.tensor_tensor(cmb[:], cmb[:], dec_em[:], op=OP.add)
        nc.sync.dma_start(out_em[:], cmb[:])

    nc.compile()


# ----------------------------------------------------------------------------
# Entry point
# ----------------------------------------------------------------------------

def _get_program(Tb, w_shapes):
    key = Tb
    if key not in _CACHE:
        import time
        t0 = time.time()
        nc = bacc.Bacc("TRN2", target_bir_lowering=False, debug=False,
                       num_devices=NCORES, num_swdge_queues=4)
        _build(nc, Tb, w_shapes)
        if os.environ.get("KERNEL_VERBOSE"):
            print(f"[kernel] build+schedule+compile: {time.time()-t0:.1f}s",
                  flush=True)
        _CACHE[key] = nc
    return _CACHE[key]


def kernel(**inputs):
    per_core, Tb = _prep(inputs["x"], inputs["edge_attr"], inputs["edge_index"])
    w = _weights_inputs(inputs)
    w_shapes = [(k, v.shape, v.dtype.type) for k, v in w.items()]
    nc = _get_program(Tb, w_shapes)

    in_maps = []
    for k in range(NCORES):
        m = dict(w)
        pc = per_core[k]
        for key in ("ea_r", "ea_em", "dm_em", "dmc_em", "gq_idx",
                    "sel", "selT", "x_r", "cnt_nm", "ind_r"):
            m[key] = pc[key]
        in_maps.append(m)

    trace = bool(int(os.environ.get("KERNEL_TRACE", "0")))
    import time as _time
    _t0 = _time.time()
    res = run_bass_kernel_spmd(
        nc, in_maps, core_ids=list(range(NCORES)), trace=trace,
        tmpdir=os.environ.get("KERNEL_TRACE_DIR") or None)
    if os.environ.get("KERNEL_VERBOSE"):
        print(f"[kernel] exec phase: {_time.time()-_t0:.1f}s", flush=True)
    if trace:
        print(f"HW exec time: {res.exec_time_ns} ns")
        if res.instructions_and_trace:
            print("trace:", res.instructions_and_trace[1])

    out = np.zeros((E, 1), dtype=np.float32)
    ET = (BLOCKS * Tb * 128) // 128
    for k in range(NCORES):
        o = res.results[k]["out_em"]           # [128, ET]
        flat = o.T.reshape(-1)                 # slot order
        orig = per_core[k]["orig"]
        valid = orig >= 0
        out[orig[valid], 0] = flat[valid]
    return out



# revision 25
# speedup vs baseline: 1.5292x; 1.1087x over previous
"""Trainium2 Bass kernel for NeuralPCG GNN message passing (8 NeuronCores).

Strategy: destination-sharded edges (core k owns all edges whose dest node is
in its 2500-node range), feature-major fp16 matmuls, dma_gather for P/Q lookups,
one-hot SEL matmuls for segment sums, one AllGather of the fused P|Q table per
message-passing step.
"""
import os
import numpy as np
import ml_dtypes
from contextlib import ExitStack

import concourse.bass as bass
import concourse.tile as tile
from concourse import bacc, mybir
from concourse.bass_utils import run_bass_kernel_spmd

N = 20000
E = 320000
L = 128
S = 3
NCORES = 8
NB = 2500            # nodes per core
BLOCKS = 20          # 128-node blocks per core
NPAD = BLOCKS * 128  # 2560
TROWS = NCORES * NPAD  # 20480 rows in the AllGathered PQ table
GT = 6               # tiles per gather group (6*128 = 768 idxs;
                     # dma_gather with num_idxs=1024 hangs the device)

F32 = mybir.dt.float32
F16 = mybir.dt.float16
F8 = mybir.dt.float8e4
I16 = mybir.dt.int16
AF = mybir.ActivationFunctionType
OP = mybir.AluOpType

NP16 = np.float16
NP8 = ml_dtypes.float8_e4m3fn

_CACHE = {}


# ----------------------------------------------------------------------------
# Host-side graph preprocessing (index manipulation + sharding only)
# ----------------------------------------------------------------------------

def _wrap_idxs(idx):
    """[n] int -> [128, n//16] int16 wrapped layout for dma_gather."""
    n = idx.shape[0]
    assert n % 16 == 0
    block = idx.reshape(n // 16, 16).T.astype(np.int16)
    return np.tile(block, (8, 1))


def _prep(x, edge_attr, edge_index):
    row = np.asarray(edge_index[0]).astype(np.int64)
    col = np.asarray(edge_index[1]).astype(np.int64)
    ea = np.asarray(edge_attr).reshape(-1).astype(np.float32)
    xf = np.asarray(x).reshape(-1).astype(np.float32)

    cnt_full = np.bincount(row, minlength=N).astype(np.float32)
    core_of = row // NB

    cores = []
    ebc_max = 0
    for k in range(NCORES):
        eids = np.nonzero(core_of == k)[0]
        order = np.argsort(row[eids], kind="stable")
        eids = eids[order]
        blk = (row[eids] - k * NB) // 128
        bc = np.bincount(blk, minlength=BLOCKS)
        ebc_max = max(ebc_max, int(bc.max()))
        cores.append((eids, blk, bc))

    Tb = max(2, (ebc_max + 127) // 128)
    EB = Tb * 128
    Epad = BLOCKS * EB
    ET = Epad // 128  # number of 128-edge tiles
    NCH = Epad // 512  # always integer: Epad = 20*Tb*128

    def trow(n):
        return (n // NB) * NPAD + (n % NB)

    per_core = []
    for k in range(NCORES):
        eids, blk, bc = cores[k]
        r = row[eids]
        c = col[eids]
        starts = np.zeros(BLOCKS, dtype=np.int64)
        np.cumsum(bc[:-1], out=starts[1:])
        pos_in_blk = np.arange(len(eids)) - starts[blk]
        dst = blk * EB + pos_in_blk

        gp = np.zeros(Epad, dtype=np.int64)
        gq = np.zeros(Epad, dtype=np.int64)
        slot = np.full(Epad, -1, dtype=np.int64)
        ea_s = np.ones(Epad, dtype=np.float32)
        dm = np.zeros(Epad, dtype=np.float32)
        orig = np.full(Epad, -1, dtype=np.int64)

        gp[dst] = trow(r)
        gq[dst] = trow(c)
        slot[dst] = (r - k * NB) % 128
        ea_s[dst] = ea[eids]
        dm[dst] = (r == c).astype(np.float32)
        orig[dst] = eids

        sel = np.zeros((Epad, 128), dtype=NP8)
        valid = slot >= 0
        sel[np.nonzero(valid)[0], slot[valid]] = NP8(1.0)
        selT = np.ascontiguousarray(sel.T)  # [128 slots, Epad]

        own = cnt_full[k * NB:(k + 1) * NB]
        tmp = np.zeros(BLOCKS * 128, dtype=np.float32)
        tmp[:NB] = own
        cnt_nm = tmp.reshape(BLOCKS, 128).T.copy()
        ind = np.zeros((1, NPAD), dtype=np.float32)
        ind[0, :NB] = (own > 0).astype(np.float32)

        x_own = np.zeros(NPAD, dtype=np.float32)
        x_own[:NB] = xf[k * NB:(k + 1) * NB]

        em = lambda a: a.reshape(ET, 128).T.copy()  # edge-slot-major [128, ET]
        per_core.append(dict(
            ea_r=ea_s.reshape(NCH, 512),
            ea_em=em(ea_s),
            dm_em=em(dm).astype(NP16),
            dmc_em=em((1.0 - dm) * (slot >= 0)).astype(NP16),
            gq_idx=_wrap_idxs(gq),
            sel=sel,
            selT=selT,
            x_r=x_own.reshape(NPAD // 512, 512),
            cnt_nm=cnt_nm,
            ind_r=ind,
            orig=orig,
        ))
    return per_core, Tb


def _weights_inputs(inp):
    """Build the weight/bias input arrays (shared across cores)."""
    g = lambda name: np.asarray(inp[name], dtype=np.float32)
    w = {}
    col = lambda a: a.reshape(128, 1).astype(np.float32)

    w["encn_W0"] = g("encn_W0").reshape(1, L)
    w["encn_b0"] = col(g("encn_b0"))
    w["encn_W1h"] = g("encn_W1").astype(NP16)
    w["encn_b1"] = col(g("encn_b1"))
    w["ence_W0"] = g("ence_W0").reshape(1, L)
    w["ence_b0"] = col(g("ence_b0"))
    w["ence_W1h"] = g("ence_W1").astype(NP16)
    w["ence_b1"] = col(g("ence_b1"))
    eW0, eb0, eW1, eb1 = g("eW0"), g("eb0"), g("eW1"), g("eb1")
    nW0, nb0, nW1, nb1 = g("nW0"), g("nb0"), g("nW1"), g("nb1")
    for s in range(S):
        w[f"eW0ab_{s}"] = np.concatenate(
            [eW0[s, :L, :], eW0[s, L:2 * L, :]], axis=1).astype(NP16)
        w[f"eW0c_{s}"] = eW0[s, 2 * L:, :].astype(NP16)
        w[f"eb0bc_{s}"] = np.tile(eb0[s].reshape(1, L), (128, 1)).astype(NP16)
        w[f"eW1h_{s}"] = eW1[s].astype(NP16)
        w[f"eb1_{s}"] = col(eb1[s])
    for s in range(S - 1):
        w[f"eW1f_{s}"] = eW1[s]
        w[f"eb1row_{s}"] = eb1[s].reshape(1, L)
        w[f"nW0a_{s}"] = nW0[s, :L, :]
        w[f"nW0bh_{s}"] = nW0[s, L:, :].astype(NP16)
        w[f"nb0_{s}"] = col(nb0[s])
        w[f"nW1h_{s}"] = nW1[s].astype(NP16)
        w[f"nb1_{s}"] = col(nb1[s])
    w["dec_W0h"] = g("dec_W0").astype(NP16)
    w["dec_b0"] = col(g("dec_b0"))
    w["dec_W1h"] = g("dec_W1").reshape(L, 1).astype(NP16)
    w["dec_b1"] = np.full((128, 1), float(np.asarray(inp["dec_b1"]).reshape(-1)[0]),
                          dtype=np.float32)
    w["ident"] = np.eye(128, dtype=np.float32)
    w["ident16"] = np.eye(128, dtype=NP16)
    return w


# ----------------------------------------------------------------------------
# Device program
# ----------------------------------------------------------------------------

def _build(nc, Tb, w_shapes):
    kb_blocks = int(os.environ.get("KB_BLOCKS", str(BLOCKS)))
    kb_no_sel = bool(int(os.environ.get("KB_NO_SEL", "0")))
    kb_no_tp = bool(int(os.environ.get("KB_NO_TP", "0")))
    kb_no_gather = bool(int(os.environ.get("KB_NO_GATHER", "0")))
    kb_no_cc = bool(int(os.environ.get("KB_NO_CC", "0")))
    kb_no_dec = bool(int(os.environ.get("KB_NO_DEC", "0")))
    EB = Tb * 128
    Epad = BLOCKS * EB
    ET = Epad // 128
    NCH = Epad // 512
    # gather groups (in tiles) per block
    groups = []
    t0 = 0
    while t0 < Tb:
        groups.append((t0, min(GT, Tb - t0)))
        t0 += GT

    din = {}

    def inp(name, shape, dtype):
        din[name] = nc.dram_tensor(name, shape, dtype, kind="ExternalInput")
        return din[name]

    inp("ea_r", [NCH, 512], F32)
    inp("ea_em", [128, ET], F32)
    inp("dm_em", [128, ET], F16)
    inp("dmc_em", [128, ET], F16)
    inp("gq_idx", [128, Epad // 16], I16)
    inp("sel", [Epad, 128], F8)
    inp("selT", [128, Epad], F8)
    inp("x_r", [NPAD // 512, 512], F32)
    inp("cnt_nm", [128, BLOCKS], F32)
    inp("ind_r", [1, NPAD], F32)
    for name, arr_shape, np_dtype in w_shapes:
        inp(name, list(arr_shape), F16 if np_dtype == NP16 else F32)

    out_em = nc.dram_tensor("out_em", [128, ET], F32, kind="ExternalOutput")

    with tile.TileContext(nc) as tc, ExitStack() as ctx:
        P = lambda name, bufs, **kw: ctx.enter_context(
            tc.tile_pool(name=name, bufs=bufs, **kw))
        const = P("const", 1)
        big = P("big", 1)
        dram = P("dram", 1, space="DRAM")
        selp = P("selp", 3)
        selTring = P("selTring", 3)
        gath = P("gath", 4)
        hring = P("hring", 6)
        hem = P("hem", 8)
        tring = P("tring", 4)
        sgring = P("sgring", 3)
        rows = P("rows", 2)
        aggring = P("aggring", 2)
        pqring = P("pqring", 2)
        ps_r = P("ps_r", 2, space="PSUM")
        ps_le = P("ps_le", 2, space="PSUM")
        ps_g = P("ps_g", 1, space="PSUM")
        ps_m = P("ps_m", 1, space="PSUM")
        ps_t = P("ps_t", 2, space="PSUM")

        # ---- load constants / weights ----
        W = {}
        for name, arr_shape, np_dtype in w_shapes:
            t = const.tile(list(arr_shape), F16 if np_dtype == NP16 else F32,
                           name=f"w_{name}")
            nc.sync.dma_start(t[:], din[name][:])
            W[name] = t
        gq_idx = const.tile([128, Epad // 16], I16, name="gq_idx_s")
        nc.sync.dma_start(gq_idx[:], din["gq_idx"][:])
        cnt = const.tile([128, BLOCKS], F32, name="cnt_s")
        nc.sync.dma_start(cnt[:], din["cnt_nm"][:])
        ind = const.tile([1, NPAD], F32, name="ind_s")
        nc.sync.dma_start(ind[:], din["ind_r"][:])
        ea_em = const.tile([128, ET], F32, name="ea_em_s")
        nc.sync.dma_start(ea_em[:], din["ea_em"][:])
        dm_em = const.tile([128, ET], F16, name="dm_em_s")
        nc.sync.dma_start(dm_em[:], din["dm_em"][:])
        dmc_em = const.tile([128, ET], F16, name="dmc_em_s")
        nc.sync.dma_start(dmc_em[:], din["dmc_em"][:])

        zeros = const.tile([128, 512], F16, name="zeros")
        nc.vector.memset(zeros[:], 0.0)
        invc = const.tile([128, BLOCKS], F32, name="invc")
        nc.vector.tensor_scalar_max(invc[:], cnt[:], 1.0)
        nc.vector.reciprocal(invc[:], invc[:])

        # ---- persistent big tensors ----
        le = big.tile([128, Epad], F16, name="le")       # edge latent (feature-major)
        ln = big.tile([128, NPAD], F32, name="ln")       # own-node latent
        sst = big.tile([128, NPAD], F32, name="sst")     # scaled segsum(h)^T
        p_own = big.tile([128, NPAD], F16, name="p_own")  # [slot, f] per block
        dec_em = big.tile([128, ET], F32, name="dec_em")
        cmb = big.tile([128, ET], F32, name="cmb")

        if kb_no_sel or kb_blocks < BLOCKS:
            nc.vector.memset(sst[:], 0.0)
        q_own = [dram.tile([NPAD, 128], F16, name=f"q_own_{s}") for s in range(S)]
        q_full = [dram.tile([TROWS, 128], F16, name=f"q_full_{s}",
                            addr_space="Shared") for s in range(S)]

        def mlp_rows(src_dram, nrows, hidden_W0, b0, W1h, b1, dst):
            """dst[:, 512j:...] = W1h.T @ relu(W0 (x) row_j + b0) + b1."""
            for j in range(nrows):
                r = rows.tile([1, 512], F32, tag="rowin")
                nc.sync.dma_start(r[:], src_dram[j:j + 1, :])
                ps = ps_r.tile([128, 512], F32, tag="r")
                nc.tensor.matmul(ps[:], hidden_W0[:], r[:], start=True, stop=True)
                h0 = hring.tile([128, 512], F16, tag="h")
                nc.scalar.activation(h0[:], ps[:], AF.Relu, bias=b0[:])
                ps2 = ps_le.tile([128, 512], F32, tag="le")
                nc.tensor.matmul(ps2[:], W1h[:], h0[:], start=True, stop=True)
                nc.scalar.activation(dst[:, 512 * j:512 * (j + 1)], ps2[:],
                                     AF.Identity, bias=b1[:])

        # ---- node encoder (edge encoder emitted after AllGather 0) ----
        mlp_rows(din["x_r"], NPAD // 512, W["encn_W0"], W["encn_b0"],
                 W["encn_W1h"], W["encn_b1"], ln)

        def pq_block(s, j):
            l16 = hem.tile([128, 128], F16, tag="hem")
            nc.scalar.activation(l16[:], ln[:, 128 * j:128 * (j + 1)], AF.Copy)
            ps = ps_t.tile([128, 256], F32, tag="tp")
            nc.tensor.matmul(ps[:], l16[:], W[f"eW0ab_{s}"][:],
                             start=True, stop=True)
            nc.vector.tensor_tensor(
                p_own[:, 128 * j:128 * (j + 1)], ps[:, 0:128],
                W[f"eb0bc_{s}"][:], op=OP.add)
            t = pqring.tile([128, 128], F16, tag="pqe")
            nc.scalar.activation(t[:], ps[:, 128:256], AF.Copy)
            nc.sync.dma_start(q_own[s][128 * j:128 * (j + 1), :], t[:])

        def pq_cc(s):
            if kb_no_cc:
                nc.sync.dma_start(q_full[s][0:NPAD, :], q_own[s][:])
            else:
                nc.gpsimd.collective_compute(
                    "AllGather", OP.bypass,
                    replica_groups=[list(range(NCORES))],
                    ins=[q_own[s].opt()],
                    outs=[q_full[s].opt()],
                )

        def node_chunk(s, j):
            """agg + node MLP for node column chunk j (512 cols)."""
            o = 512 * j
            a_ps = ps_t.tile([128, 512], F32, tag="tp")
            nc.tensor.matmul(a_ps[:], W[f"eW1f_{s}"][:], sst[:, o:o + 512],
                             start=True, stop=False)
            nc.tensor.matmul(a_ps[:], W[f"eb1row_{s}"][:],
                             ind[:, o:o + 512], start=False, stop=True)
            agg = aggring.tile([128, 512], F16, tag="agg16")
            nc.scalar.activation(agg[:], a_ps[:], AF.Copy)
            p_ps = ps_r.tile([128, 512], F32, tag="r")
            nc.tensor.matmul(p_ps[:], W[f"nW0a_{s}"][:], ln[:, o:o + 512],
                             start=True, stop=False)
            nc.tensor.matmul(p_ps[:], W[f"nW0bh_{s}"][:], agg[:],
                             start=False, stop=True)
            hn = hring.tile([128, 512], F16, tag="h")
            nc.scalar.activation(hn[:], p_ps[:], AF.Relu,
                                 bias=W[f"nb0_{s}"][:])
            l_ps = ps_le.tile([128, 512], F32, tag="le")
            nc.tensor.matmul(l_ps[:], W[f"nW1h_{s}"][:], hn[:],
                             start=True, stop=True)
            nc.scalar.activation(ln[:, o:o + 512], l_ps[:],
                                 AF.Identity, bias=W[f"nb1_{s}"][:])

        for j in range(BLOCKS):
            pq_block(0, j)
        pq_cc(0)
        mlp_rows(din["ea_r"], NCH, W["ence_W0"], W["ence_b0"],
                 W["ence_W1h"], W["ence_b1"], le)

        g_ctr = 0
        for s in range(S):
            do_agg = s < S - 1
            eW0c = W[f"eW0c_{s}"]
            eW1h = W[f"eW1h_{s}"]
            eb1 = W[f"eb1_{s}"]
            for b in range(kb_blocks):
                selT_b = selTring.tile([128, EB], F8, tag="selT")
                nc.sync.dma_start(selT_b[:], din["selT"][:, b * EB:(b + 1) * EB])
                pob = p_own[:, 128 * b:128 * (b + 1)]
                if do_agg and not kb_no_sel:
                    g_ps = ps_g.tile([128, 128], F32, tag="g")
                for (gt0, gnt) in groups:
                    # gather this group's Q rows
                    i0 = b * EB + gt0 * 128
                    ni = gnt * 128
                    gq_t = gath.tile([128, 1, GT * 128], F16, tag="gq")
                    if kb_no_gather:
                        nc.vector.memset(gq_t[:], 0.0)
                    else:
                        nc.gpsimd.dma_gather(
                            gq_t[:, :, :ni], q_full[s][:, :],
                            gq_idx[:, i0 // 16:(i0 + ni) // 16],
                            num_idxs=ni, num_idxs_reg=ni,
                            elem_size=128, elem_step=128, transpose=True,
                            queue_num=g_ctr % 4)
                        g_ctr += 1
                    if do_agg and not kb_no_sel:
                        sel_t = selp.tile([128, GT, 128], F8, tag="sel")
                        nc.sync.dma_start(
                            sel_t[:, :gnt, :],
                            din["sel"][i0:i0 + ni, :].rearrange(
                                "(t p) s -> p t s", p=128))
                    # chunks of <=512 within the group
                    co = 0
                    while co < ni:
                        cw = min(512, ni - co)
                        goff = i0 + co            # global edge-slot offset
                        boff = gt0 * 128 + co     # offset within the block
                        ps = ps_r.tile([128, 512], F32, tag="r")
                        nc.tensor.matmul(ps[:, :cw], pob,
                                         selT_b[:, boff:boff + cw],
                                         start=True, stop=False)
                        nc.tensor.matmul(ps[:, :cw], eW0c[:],
                                         le[:, goff:goff + cw],
                                         start=False, stop=True)
                        t2 = tring.tile([128, 512], F16, tag="t")
                        nc.vector.tensor_tensor(t2[:, :cw],
                                                gq_t[:, 0, co:co + cw],
                                                ps[:, :cw], op=OP.add)
                        h = hring.tile([128, 512], F16, tag="h")
                        nc.vector.tensor_tensor(h[:, :cw], t2[:, :cw],
                                                zeros[:, :cw], op=OP.max)
                        ps2 = ps_le.tile([128, 512], F32, tag="le")
                        nc.tensor.matmul(ps2[:, :cw], eW1h[:], h[:, :cw],
                                         start=True, stop=True)
                        nc.scalar.activation(le[:, goff:goff + cw], ps2[:, :cw],
                                             AF.Identity, bias=eb1[:])
                        if do_agg and not kb_no_sel:
                            for u in range(cw // 128):
                                tt = gt0 + (co // 128) + u
                                he = hem.tile([128, 128], F16, tag="hem")
                                if kb_no_tp:
                                    nc.vector.tensor_copy(
                                        he[:], h[:, 128 * u:128 * (u + 1)])
                                else:
                                    ht_ps = ps_t.tile([128, 128], F16, tag="tp")
                                    nc.tensor.transpose(
                                        ht_ps[:], h[:, 128 * u:128 * (u + 1)],
                                        W["ident16"][:])
                                    if tt % 2 == 0:
                                        nc.scalar.activation(he[:], ht_ps[:],
                                                             AF.Copy)
                                    else:
                                        nc.vector.tensor_tensor(
                                            he[:], ht_ps[:], zeros[:, 0:128],
                                            op=OP.add)
                                nc.tensor.matmul(
                                    g_ps[:], sel_t[:, (co // 128) + u, :], he[:],
                                    start=(tt == 0), stop=(tt == Tb - 1),
                                    skip_group_check=True)
                        co += cw
                if do_agg and not kb_no_sel:
                    sg = sgring.tile([128, 128], F32, tag="sg")
                    nc.vector.tensor_tensor(sg[:], g_ps[:],
                                            invc[:, b:b + 1].to_broadcast(
                                                [128, 128]), op=OP.mult)
                    ps_tr = ps_m.tile([128, 128], F32, tag="m")
                    nc.tensor.transpose(ps_tr[:], sg[:], W["ident"][:])
                    nc.vector.tensor_tensor(sst[:, 128 * b:128 * (b + 1)],
                                            ps_tr[:], zeros[:, 0:128], op=OP.add)
                    if b % 4 == 3 and kb_blocks == BLOCKS:
                        node_chunk(s, b // 4)
                        for jb in range(4 * (b // 4), 4 * (b // 4) + 4):
                            pq_block(s + 1, jb)

            if do_agg:
                pq_cc(s + 1)

        # ---- decoder ----
        for ci in range(0 if kb_no_dec else NCH):
            off = 512 * ci
            ps = ps_r.tile([128, 512], F32, tag="r")
            nc.tensor.matmul(ps[:], W["dec_W0h"][:], le[:, off:off + 512],
                             start=True, stop=True)
            hd = hring.tile([128, 512], F16, tag="h")
            nc.scalar.activation(hd[:], ps[:], AF.Relu, bias=W["dec_b0"][:])
            d_ps = ps_m.tile([128, 4], F32, tag="m")
            for u in range(4):
                nc.tensor.matmul(d_ps[:, u:u + 1], hd[:, 128 * u:128 * (u + 1)],
                                 W["dec_W1h"][:], start=True, stop=True)
            nc.vector.tensor_tensor(dec_em[:, 4 * ci:4 * ci + 4], d_ps[:],
                                    W["dec_b1"][:].to_broadcast([128, 4]),
                                    op=OP.add)

        # ---- final combine: out = dm*0.5*sqrt(ea) + dmc*dec ----
        nc.scalar.sqrt(cmb[:], ea_em[:])
        nc.vector.scalar_tensor_tensor(cmb[:], dm_em[:], 0.5, cmb[:],
                                       op0=OP.mult, op1=OP.mult)
        nc.vector.tensor_tensor(dec_em[:], dmc_em[:], dec_em[:], op=OP.mult)
        nc.vector.tensor_tensor(cmb[:], cmb[:], dec_em[:], op=OP.add)
        nc.sync.dma_start(out_em[:], cmb[:])

    nc.compile()


# ----------------------------------------------------------------------------
# Entry point
# ----------------------------------------------------------------------------

def _get_program(Tb, w_shapes):
    key = Tb
    if key not in _CACHE:
        import time
        t0 = time.time()
        nc = bacc.Bacc("TRN2", target_bir_lowering=False, debug=False,
                       num_devices=NCORES, num_swdge_queues=4)
        _build(nc, Tb, w_shapes)
        if os.environ.get("KERNEL_VERBOSE"):
            print(f"[kernel] build+schedule+compile: {time.time()-t0:.1f}s",
                  flush=True)
        _CACHE[key] = nc
    return _CACHE[key]


def kernel(**inputs):
    per_core, Tb = _prep(inputs["x"], inputs["edge_attr"], inputs["edge_index"])
    w = _weights_inputs(inputs)
    w_shapes = [(k, v.shape, v.dtype.type) for k, v in w.items()]
    nc = _get_program(Tb, w_shapes)

    in_maps = []
    for k in range(NCORES):
        m = dict(w)
        pc = per_core[k]
        for key in ("ea_r", "ea_em", "dm_em", "dmc_em", "gq_idx",
                    "sel", "selT", "x_r", "cnt_nm", "ind_r"):
            m[key] = pc[key]
        in_maps.append(m)

    trace = bool(int(os.environ.get("KERNEL_TRACE", "0")))
    import time as _time
    _t0 = _time.time()
    res = run_bass_kernel_spmd(
        nc, in_maps, core_ids=list(range(NCORES)), trace=trace,
        tmpdir=os.environ.get("KERNEL_TRACE_DIR") or None)
    if os.environ.get("KERNEL_VERBOSE"):
        print(f"[kernel] exec phase: {_time.time()-_t0:.1f}s", flush=True)
    if trace:
        print(f"HW exec time: {res.exec_time_ns} ns")
        if res.instructions_and_trace:
            print("trace:", res.instructions_and_trace[1])

    out = np.zeros((E, 1), dtype=np.float32)
    ET = (BLOCKS * Tb * 128) // 128
    for k in range(NCORES):
        o = res.results[k]["out_em"]           # [128, ET]
        flat = o.T.reshape(-1)                 # slot order
        orig = per_core[k]["orig"]
        valid = orig >= 0
        out[orig[valid], 0] = flat[valid]
    return out



# revision 26
# speedup vs baseline: 1.6147x; 1.0559x over previous
"""Trainium2 Bass kernel for NeuralPCG GNN message passing (8 NeuronCores).

Strategy: destination-sharded edges (core k owns all edges whose dest node is
in its 2500-node range), feature-major fp16 matmuls, dma_gather for P/Q lookups,
one-hot SEL matmuls for segment sums, one AllGather of the fused P|Q table per
message-passing step.
"""
import os
import numpy as np
import ml_dtypes
from contextlib import ExitStack

import concourse.bass as bass
import concourse.tile as tile
from concourse import bacc, mybir
from concourse.bass_utils import run_bass_kernel_spmd

N = 20000
E = 320000
L = 128
S = 3
NCORES = 8
NB = 2500            # nodes per core
BLOCKS = 20          # 128-node blocks per core
NPAD = BLOCKS * 128  # 2560
TROWS = NCORES * NPAD  # 20480 rows in the AllGathered PQ table
GT = 6               # tiles per gather group (6*128 = 768 idxs;
                     # dma_gather with num_idxs=1024 hangs the device)

F32 = mybir.dt.float32
F16 = mybir.dt.float16
F8 = mybir.dt.float8e4
I16 = mybir.dt.int16
AF = mybir.ActivationFunctionType
OP = mybir.AluOpType

NP16 = np.float16
NP8 = ml_dtypes.float8_e4m3fn

_CACHE = {}


# ----------------------------------------------------------------------------
# Host-side graph preprocessing (index manipulation + sharding only)
# ----------------------------------------------------------------------------

def _wrap_idxs(idx):
    """[n] int -> [128, n//16] int16 wrapped layout for dma_gather."""
    n = idx.shape[0]
    assert n % 16 == 0
    block = idx.reshape(n // 16, 16).T.astype(np.int16)
    return np.tile(block, (8, 1))


def _prep(x, edge_attr, edge_index):
    row = np.asarray(edge_index[0]).astype(np.int64)
    col = np.asarray(edge_index[1]).astype(np.int64)
    ea = np.asarray(edge_attr).reshape(-1).astype(np.float32)
    xf = np.asarray(x).reshape(-1).astype(np.float32)

    cnt_full = np.bincount(row, minlength=N).astype(np.float32)
    core_of = row // NB

    cores = []
    ebc_max = 0
    for k in range(NCORES):
        eids = np.nonzero(core_of == k)[0]
        order = np.argsort(row[eids], kind="stable")
        eids = eids[order]
        blk = (row[eids] - k * NB) // 128
        bc = np.bincount(blk, minlength=BLOCKS)
        ebc_max = max(ebc_max, int(bc.max()))
        cores.append((eids, blk, bc))

    Tb = max(2, (ebc_max + 127) // 128)
    EB = Tb * 128
    Epad = BLOCKS * EB
    ET = Epad // 128  # number of 128-edge tiles
    NCH = Epad // 512  # always integer: Epad = 20*Tb*128

    def trow(n):
        return (n // NB) * NPAD + (n % NB)

    per_core = []
    for k in range(NCORES):
        eids, blk, bc = cores[k]
        r = row[eids]
        c = col[eids]
        starts = np.zeros(BLOCKS, dtype=np.int64)
        np.cumsum(bc[:-1], out=starts[1:])
        pos_in_blk = np.arange(len(eids)) - starts[blk]
        dst = blk * EB + pos_in_blk

        gp = np.zeros(Epad, dtype=np.int64)
        gq = np.zeros(Epad, dtype=np.int64)
        slot = np.full(Epad, -1, dtype=np.int64)
        ea_s = np.ones(Epad, dtype=np.float32)
        dm = np.zeros(Epad, dtype=np.float32)
        orig = np.full(Epad, -1, dtype=np.int64)

        gp[dst] = trow(r)
        gq[dst] = trow(c)
        slot[dst] = (r - k * NB) % 128
        ea_s[dst] = ea[eids]
        dm[dst] = (r == c).astype(np.float32)
        orig[dst] = eids

        sel = np.zeros((Epad, 128), dtype=NP8)
        valid = slot >= 0
        sel[np.nonzero(valid)[0], slot[valid]] = NP8(1.0)
        selT = np.ascontiguousarray(sel.T)  # [128 slots, Epad]

        own = cnt_full[k * NB:(k + 1) * NB]
        tmp = np.zeros(BLOCKS * 128, dtype=np.float32)
        tmp[:NB] = own
        cnt_nm = tmp.reshape(BLOCKS, 128).T.copy()
        ind = np.zeros((1, NPAD), dtype=np.float32)
        ind[0, :NB] = (own > 0).astype(np.float32)

        x_own = np.zeros(NPAD, dtype=np.float32)
        x_own[:NB] = xf[k * NB:(k + 1) * NB]

        em = lambda a: a.reshape(ET, 128).T.copy()  # edge-slot-major [128, ET]
        per_core.append(dict(
            ea_r=ea_s.reshape(NCH, 512),
            ea_em=em(ea_s),
            dm_em=em(dm).astype(NP16),
            dmc_em=em((1.0 - dm) * (slot >= 0)).astype(NP16),
            gq_idx=_wrap_idxs(gq),
            sel=sel,
            selT=selT,
            x_r=x_own.reshape(NPAD // 512, 512),
            cnt_nm=cnt_nm,
            ind_r=ind,
            orig=orig,
        ))
    return per_core, Tb


def _weights_inputs(inp):
    """Build the weight/bias input arrays (shared across cores)."""
    g = lambda name: np.asarray(inp[name], dtype=np.float32)
    w = {}
    col = lambda a: a.reshape(128, 1).astype(np.float32)

    w["encn_W0"] = g("encn_W0").reshape(1, L)
    w["encn_b0"] = col(g("encn_b0"))
    w["encn_W1h"] = g("encn_W1").astype(NP16)
    w["encn_b1"] = col(g("encn_b1"))
    w["ence_W0"] = g("ence_W0").reshape(1, L)
    w["ence_b0"] = col(g("ence_b0"))
    w["ence_W1h"] = g("ence_W1").astype(NP16)
    w["ence_b1"] = col(g("ence_b1"))
    eW0, eb0, eW1, eb1 = g("eW0"), g("eb0"), g("eW1"), g("eb1")
    nW0, nb0, nW1, nb1 = g("nW0"), g("nb0"), g("nW1"), g("nb1")
    for s in range(S):
        w[f"eW0ab_{s}"] = np.concatenate(
            [eW0[s, :L, :], eW0[s, L:2 * L, :]], axis=1).astype(NP16)
        w[f"eW0c_{s}"] = eW0[s, 2 * L:, :].astype(NP16)
        w[f"eb0bc_{s}"] = np.tile(eb0[s].reshape(1, L), (128, 1)).astype(NP16)
        w[f"eW1h_{s}"] = eW1[s].astype(NP16)
        w[f"eb1_{s}"] = col(eb1[s])
    for s in range(S - 1):
        w[f"eW1f_{s}"] = eW1[s]
        w[f"eb1row_{s}"] = eb1[s].reshape(1, L)
        w[f"nW0a_{s}"] = nW0[s, :L, :]
        w[f"nW0bh_{s}"] = nW0[s, L:, :].astype(NP16)
        w[f"nb0_{s}"] = col(nb0[s])
        w[f"nW1h_{s}"] = nW1[s].astype(NP16)
        w[f"nb1_{s}"] = col(nb1[s])
    w["dec_W0h"] = g("dec_W0").astype(NP16)
    w["dec_b0"] = col(g("dec_b0"))
    w["dec_W1h"] = g("dec_W1").reshape(L, 1).astype(NP16)
    w["dec_b1"] = np.full((128, 1), float(np.asarray(inp["dec_b1"]).reshape(-1)[0]),
                          dtype=np.float32)
    w["ident"] = np.eye(128, dtype=np.float32)
    w["ident16"] = np.eye(128, dtype=NP16)
    return w


# ----------------------------------------------------------------------------
# Device program
# ----------------------------------------------------------------------------

def _build(nc, Tb, w_shapes):
    kb_blocks = int(os.environ.get("KB_BLOCKS", str(BLOCKS)))
    kb_no_sel = bool(int(os.environ.get("KB_NO_SEL", "0")))
    kb_no_tp = bool(int(os.environ.get("KB_NO_TP", "0")))
    kb_no_gather = bool(int(os.environ.get("KB_NO_GATHER", "0")))
    kb_no_cc = bool(int(os.environ.get("KB_NO_CC", "0")))
    kb_no_dec = bool(int(os.environ.get("KB_NO_DEC", "0")))
    EB = Tb * 128
    Epad = BLOCKS * EB
    ET = Epad // 128
    NCH = Epad // 512
    # gather groups (in tiles) per block
    groups = []
    t0 = 0
    while t0 < Tb:
        groups.append((t0, min(GT, Tb - t0)))
        t0 += GT

    din = {}

    def inp(name, shape, dtype):
        din[name] = nc.dram_tensor(name, shape, dtype, kind="ExternalInput")
        return din[name]

    inp("ea_r", [NCH, 512], F32)
    inp("ea_em", [128, ET], F32)
    inp("dm_em", [128, ET], F16)
    inp("dmc_em", [128, ET], F16)
    inp("gq_idx", [128, Epad // 16], I16)
    inp("sel", [Epad, 128], F8)
    inp("selT", [128, Epad], F8)
    inp("x_r", [NPAD // 512, 512], F32)
    inp("cnt_nm", [128, BLOCKS], F32)
    inp("ind_r", [1, NPAD], F32)
    for name, arr_shape, np_dtype in w_shapes:
        inp(name, list(arr_shape), F16 if np_dtype == NP16 else F32)

    out_em = nc.dram_tensor("out_em", [128, ET], F32, kind="ExternalOutput")

    with tile.TileContext(nc) as tc, ExitStack() as ctx:
        P = lambda name, bufs, **kw: ctx.enter_context(
            tc.tile_pool(name=name, bufs=bufs, **kw))
        const = P("const", 1)
        big = P("big", 1)
        dram = P("dram", 1, space="DRAM")
        selp = P("selp", 3)
        selTring = P("selTring", 3)
        gath = P("gath", 6)
        hring = P("hring", 6)
        hem = P("hem", 8)
        tring = P("tring", 4)
        sgring = P("sgring", 3)
        rows = P("rows", 2)
        aggring = P("aggring", 2)
        pqring = P("pqring", 2)
        ps_r = P("ps_r", 2, space="PSUM")
        ps_le = P("ps_le", 2, space="PSUM")
        ps_g = P("ps_g", 1, space="PSUM")
        ps_m = P("ps_m", 1, space="PSUM")
        ps_t = P("ps_t", 2, space="PSUM")

        # ---- load constants / weights ----
        W = {}
        for name, arr_shape, np_dtype in w_shapes:
            t = const.tile(list(arr_shape), F16 if np_dtype == NP16 else F32,
                           name=f"w_{name}")
            nc.sync.dma_start(t[:], din[name][:])
            W[name] = t
        gq_idx = const.tile([128, Epad // 16], I16, name="gq_idx_s")
        nc.sync.dma_start(gq_idx[:], din["gq_idx"][:])
        cnt = const.tile([128, BLOCKS], F32, name="cnt_s")
        nc.sync.dma_start(cnt[:], din["cnt_nm"][:])
        ind = const.tile([1, NPAD], F32, name="ind_s")
        nc.sync.dma_start(ind[:], din["ind_r"][:])
        ea_em = const.tile([128, ET], F32, name="ea_em_s")
        nc.sync.dma_start(ea_em[:], din["ea_em"][:])
        dm_em = const.tile([128, ET], F16, name="dm_em_s")
        nc.sync.dma_start(dm_em[:], din["dm_em"][:])
        dmc_em = const.tile([128, ET], F16, name="dmc_em_s")
        nc.sync.dma_start(dmc_em[:], din["dmc_em"][:])

        zeros = const.tile([128, 512], F16, name="zeros")
        nc.vector.memset(zeros[:], 0.0)
        invc = const.tile([128, BLOCKS], F32, name="invc")
        nc.vector.tensor_scalar_max(invc[:], cnt[:], 1.0)
        nc.vector.reciprocal(invc[:], invc[:])

        # ---- persistent big tensors ----
        le = big.tile([128, Epad], F16, name="le")       # edge latent (feature-major)
        ln = big.tile([128, NPAD], F32, name="ln")       # own-node latent
        sst = big.tile([128, NPAD], F32, name="sst")     # scaled segsum(h)^T
        p_own = big.tile([128, NPAD], F16, name="p_own")  # [slot, f] per block
        dec_em = big.tile([128, ET], F32, name="dec_em")
        cmb = big.tile([128, ET], F32, name="cmb")

        if kb_no_sel or kb_blocks < BLOCKS:
            nc.vector.memset(sst[:], 0.0)
        q_own = [dram.tile([NPAD, 128], F16, name=f"q_own_{s}") for s in range(S)]
        q_full = [dram.tile([TROWS, 128], F16, name=f"q_full_{s}",
                            addr_space="Shared") for s in range(S)]

        def mlp_rows(src_dram, nrows, hidden_W0, b0, W1h, b1, dst, j0=0):
            """dst[:, 512j:...] = W1h.T @ relu(W0 (x) row_j + b0) + b1."""
            for j in range(j0, nrows):
                r = rows.tile([1, 512], F32, tag="rowin")
                nc.sync.dma_start(r[:], src_dram[j:j + 1, :])
                ps = ps_r.tile([128, 512], F32, tag="r")
                nc.tensor.matmul(ps[:], hidden_W0[:], r[:], start=True, stop=True)
                h0 = hring.tile([128, 512], F16, tag="h")
                nc.scalar.activation(h0[:], ps[:], AF.Relu, bias=b0[:])
                ps2 = ps_le.tile([128, 512], F32, tag="le")
                nc.tensor.matmul(ps2[:], W1h[:], h0[:], start=True, stop=True)
                nc.scalar.activation(dst[:, 512 * j:512 * (j + 1)], ps2[:],
                                     AF.Identity, bias=b1[:])

        # ---- node encoder (edge encoder emitted after AllGather 0) ----
        mlp_rows(din["x_r"], NPAD // 512, W["encn_W0"], W["encn_b0"],
                 W["encn_W1h"], W["encn_b1"], ln)

        def pq_block(s, j):
            l16 = hem.tile([128, 128], F16, tag="hem")
            nc.scalar.activation(l16[:], ln[:, 128 * j:128 * (j + 1)], AF.Copy)
            ps = ps_t.tile([128, 256], F32, tag="tp")
            nc.tensor.matmul(ps[:], l16[:], W[f"eW0ab_{s}"][:],
                             start=True, stop=True)
            nc.vector.tensor_tensor(
                p_own[:, 128 * j:128 * (j + 1)], ps[:, 0:128],
                W[f"eb0bc_{s}"][:], op=OP.add)
            t = pqring.tile([128, 128], F16, tag="pqe")
            nc.scalar.activation(t[:], ps[:, 128:256], AF.Copy)
            nc.sync.dma_start(q_own[s][128 * j:128 * (j + 1), :], t[:])

        def pq_cc(s):
            if kb_no_cc:
                nc.sync.dma_start(q_full[s][0:NPAD, :], q_own[s][:])
            else:
                nc.gpsimd.collective_compute(
                    "AllGather", OP.bypass,
                    replica_groups=[list(range(NCORES))],
                    ins=[q_own[s].opt()],
                    outs=[q_full[s].opt()],
                )

        def node_chunk(s, j):
            """agg + node MLP for node column chunk j (512 cols)."""
            o = 512 * j
            a_ps = ps_t.tile([128, 512], F32, tag="tp")
            nc.tensor.matmul(a_ps[:], W[f"eW1f_{s}"][:], sst[:, o:o + 512],
                             start=True, stop=False)
            nc.tensor.matmul(a_ps[:], W[f"eb1row_{s}"][:],
                             ind[:, o:o + 512], start=False, stop=True)
            agg = aggring.tile([128, 512], F16, tag="agg16")
            nc.scalar.activation(agg[:], a_ps[:], AF.Copy)
            p_ps = ps_r.tile([128, 512], F32, tag="r")
            nc.tensor.matmul(p_ps[:], W[f"nW0a_{s}"][:], ln[:, o:o + 512],
                             start=True, stop=False)
            nc.tensor.matmul(p_ps[:], W[f"nW0bh_{s}"][:], agg[:],
                             start=False, stop=True)
            hn = hring.tile([128, 512], F16, tag="h")
            nc.scalar.activation(hn[:], p_ps[:], AF.Relu,
                                 bias=W[f"nb0_{s}"][:])
            l_ps = ps_le.tile([128, 512], F32, tag="le")
            nc.tensor.matmul(l_ps[:], W[f"nW1h_{s}"][:], hn[:],
                             start=True, stop=True)
            nc.scalar.activation(ln[:, o:o + 512], l_ps[:],
                                 AF.Identity, bias=W[f"nb1_{s}"][:])

        for j in range(BLOCKS):
            pq_block(0, j)
        pq_cc(0)

        enc_done = 0

        def enc_upto(c_end):
            nonlocal enc_done
            if c_end > enc_done:
                mlp_rows(din["ea_r"], c_end, W["ence_W0"], W["ence_b0"],
                         W["ence_W1h"], W["ence_b1"], le, j0=enc_done)
                enc_done = c_end

        g_ctr = 0
        for s in range(S):
            do_agg = s < S - 1
            eW0c = W[f"eW0c_{s}"]
            eW1h = W[f"eW1h_{s}"]
            eb1 = W[f"eb1_{s}"]
            for b in range(kb_blocks):
                if s == 0:
                    enc_upto(min(NCH, ((b + 1) * EB + 511) // 512))
                selT_b = selTring.tile([128, EB], F8, tag="selT")
                nc.sync.dma_start(selT_b[:], din["selT"][:, b * EB:(b + 1) * EB])
                pob = p_own[:, 128 * b:128 * (b + 1)]
                if do_agg and not kb_no_sel:
                    g_ps = ps_g.tile([128, 128], F32, tag="g")
                for (gt0, gnt) in groups:
                    # gather this group's Q rows
                    i0 = b * EB + gt0 * 128
                    ni = gnt * 128
                    gq_t = gath.tile([128, 1, GT * 128], F16, tag="gq")
                    if kb_no_gather:
                        nc.vector.memset(gq_t[:], 0.0)
                    else:
                        nc.gpsimd.dma_gather(
                            gq_t[:, :, :ni], q_full[s][:, :],
                            gq_idx[:, i0 // 16:(i0 + ni) // 16],
                            num_idxs=ni, num_idxs_reg=ni,
                            elem_size=128, elem_step=128, transpose=True,
                            queue_num=g_ctr % 4)
                        g_ctr += 1
                    if do_agg and not kb_no_sel:
                        sel_t = selp.tile([128, GT, 128], F8, tag="sel")
                        nc.sync.dma_start(
                            sel_t[:, :gnt, :],
                            din["sel"][i0:i0 + ni, :].rearrange(
                                "(t p) s -> p t s", p=128))
                    # chunks of <=512 within the group
                    co = 0
                    while co < ni:
                        cw = min(512, ni - co)
                        goff = i0 + co            # global edge-slot offset
                        boff = gt0 * 128 + co     # offset within the block
                        ps = ps_r.tile([128, 512], F32, tag="r")
                        nc.tensor.matmul(ps[:, :cw], pob,
                                         selT_b[:, boff:boff + cw],
                                         start=True, stop=False)
                        nc.tensor.matmul(ps[:, :cw], eW0c[:],
                                         le[:, goff:goff + cw],
                                         start=False, stop=True)
                        t2 = tring.tile([128, 512], F16, tag="t")
                        nc.vector.tensor_tensor(t2[:, :cw],
                                                gq_t[:, 0, co:co + cw],
                                                ps[:, :cw], op=OP.add)
                        h = hring.tile([128, 512], F16, tag="h")
                        nc.vector.tensor_tensor(h[:, :cw], t2[:, :cw],
                                                zeros[:, :cw], op=OP.max)
                        ps2 = ps_le.tile([128, 512], F32, tag="le")
                        nc.tensor.matmul(ps2[:, :cw], eW1h[:], h[:, :cw],
                                         start=True, stop=True)
                        nc.scalar.activation(le[:, goff:goff + cw], ps2[:, :cw],
                                             AF.Identity, bias=eb1[:])
                        if do_agg and not kb_no_sel:
                            for u in range(cw // 128):
                                tt = gt0 + (co // 128) + u
                                he = hem.tile([128, 128], F16, tag="hem")
                                if kb_no_tp:
                                    nc.vector.tensor_copy(
                                        he[:], h[:, 128 * u:128 * (u + 1)])
                                else:
                                    ht_ps = ps_t.tile([128, 128], F16, tag="tp")
                                    nc.tensor.transpose(
                                        ht_ps[:], h[:, 128 * u:128 * (u + 1)],
                                        W["ident16"][:])
                                    if tt % 2 == 0:
                                        nc.scalar.activation(he[:], ht_ps[:],
                                                             AF.Copy)
                                    else:
                                        nc.vector.tensor_tensor(
                                            he[:], ht_ps[:], zeros[:, 0:128],
                                            op=OP.add)
                                nc.tensor.matmul(
                                    g_ps[:], sel_t[:, (co // 128) + u, :], he[:],
                                    start=(tt == 0), stop=(tt == Tb - 1),
                                    skip_group_check=True)
                        co += cw
                if do_agg and not kb_no_sel:
                    sg = sgring.tile([128, 128], F32, tag="sg")
                    nc.vector.tensor_tensor(sg[:], g_ps[:],
                                            invc[:, b:b + 1].to_broadcast(
                                                [128, 128]), op=OP.mult)
                    ps_tr = ps_m.tile([128, 128], F32, tag="m")
                    nc.tensor.transpose(ps_tr[:], sg[:], W["ident"][:])
                    nc.vector.tensor_tensor(sst[:, 128 * b:128 * (b + 1)],
                                            ps_tr[:], zeros[:, 0:128], op=OP.add)
                    if b % 4 == 3 and kb_blocks == BLOCKS:
                        node_chunk(s, b // 4)
                        for jb in range(4 * (b // 4), 4 * (b // 4) + 4):
                            pq_block(s + 1, jb)

            if do_agg:
                pq_cc(s + 1)

        # ---- decoder ----
        for ci in range(0 if kb_no_dec else NCH):
            off = 512 * ci
            ps = ps_r.tile([128, 512], F32, tag="r")
            nc.tensor.matmul(ps[:], W["dec_W0h"][:], le[:, off:off + 512],
                             start=True, stop=True)
            hd = hring.tile([128, 512], F16, tag="h")
            nc.scalar.activation(hd[:], ps[:], AF.Relu, bias=W["dec_b0"][:])
            d_ps = ps_m.tile([128, 4], F32, tag="m")
            for u in range(4):
                nc.tensor.matmul(d_ps[:, u:u + 1], hd[:, 128 * u:128 * (u + 1)],
                                 W["dec_W1h"][:], start=True, stop=True)
            nc.vector.tensor_tensor(dec_em[:, 4 * ci:4 * ci + 4], d_ps[:],
                                    W["dec_b1"][:].to_broadcast([128, 4]),
                                    op=OP.add)

        # ---- final combine: out = dm*0.5*sqrt(ea) + dmc*dec ----
        nc.scalar.sqrt(cmb[:], ea_em[:])
        nc.vector.scalar_tensor_tensor(cmb[:], dm_em[:], 0.5, cmb[:],
                                       op0=OP.mult, op1=OP.mult)
        nc.vector.tensor_tensor(dec_em[:], dmc_em[:], dec_em[:], op=OP.mult)
        nc.vector.tensor_tensor(cmb[:], cmb[:], dec_em[:], op=OP.add)
        nc.sync.dma_start(out_em[:], cmb[:])

    nc.compile()


# ----------------------------------------------------------------------------
# Entry point
# ----------------------------------------------------------------------------

def _get_program(Tb, w_shapes):
    key = Tb
    if key not in _CACHE:
        import time
        t0 = time.time()
        nc = bacc.Bacc("TRN2", target_bir_lowering=False, debug=False,
                       num_devices=NCORES, num_swdge_queues=4)
        _build(nc, Tb, w_shapes)
        if os.environ.get("KERNEL_VERBOSE"):
            print(f"[kernel] build+schedule+compile: {time.time()-t0:.1f}s",
                  flush=True)
        _CACHE[key] = nc
    return _CACHE[key]


def kernel(**inputs):
    per_core, Tb = _prep(inputs["x"], inputs["edge_attr"], inputs["edge_index"])
    w = _weights_inputs(inputs)
    w_shapes = [(k, v.shape, v.dtype.type) for k, v in w.items()]
    nc = _get_program(Tb, w_shapes)

    in_maps = []
    for k in range(NCORES):
        m = dict(w)
        pc = per_core[k]
        for key in ("ea_r", "ea_em", "dm_em", "dmc_em", "gq_idx",
                    "sel", "selT", "x_r", "cnt_nm", "ind_r"):
            m[key] = pc[key]
        in_maps.append(m)

    trace = bool(int(os.environ.get("KERNEL_TRACE", "0")))
    import time as _time
    _t0 = _time.time()
    res = run_bass_kernel_spmd(
        nc, in_maps, core_ids=list(range(NCORES)), trace=trace,
        tmpdir=os.environ.get("KERNEL_TRACE_DIR") or None)
    if os.environ.get("KERNEL_VERBOSE"):
        print(f"[kernel] exec phase: {_time.time()-_t0:.1f}s", flush=True)
    if trace:
        print(f"HW exec time: {res.exec_time_ns} ns")
        if res.instructions_and_trace:
            print("trace:", res.instructions_and_trace[1])

    out = np.zeros((E, 1), dtype=np.float32)
    ET = (BLOCKS * Tb * 128) // 128
    for k in range(NCORES):
        o = res.results[k]["out_em"]           # [128, ET]
        flat = o.T.reshape(-1)                 # slot order
        orig = per_core[k]["orig"]
        valid = orig >= 0
        out[orig[valid], 0] = flat[valid]
    return out



# revision 35
# speedup vs baseline: 1.6610x; 1.0287x over previous
"""Trainium2 Bass kernel for NeuralPCG GNN message passing (8 NeuronCores).

Strategy: destination-sharded edges (core k owns all edges whose dest node is
in its 2500-node range), feature-major fp16 matmuls, dma_gather for P/Q lookups,
one-hot SEL matmuls for segment sums, one AllGather of the fused P|Q table per
message-passing step.
"""
import os
import numpy as np
import ml_dtypes
from contextlib import ExitStack

import concourse.bass as bass
import concourse.tile as tile
from concourse import bacc, mybir
from concourse.bass_utils import run_bass_kernel_spmd

N = 20000
E = 320000
L = 128
S = 3
NCORES = 8
NB = 2500            # nodes per core
BLOCKS = 20          # 128-node blocks per core
NPAD = BLOCKS * 128  # 2560
TROWS = NCORES * NPAD  # 20480 rows in the AllGathered PQ table
GT = 6               # tiles per gather group (6*128 = 768 idxs;
                     # dma_gather with num_idxs=1024 hangs the device)

F32 = mybir.dt.float32
F16 = mybir.dt.float16
F8 = mybir.dt.float8e4
I16 = mybir.dt.int16
AF = mybir.ActivationFunctionType
OP = mybir.AluOpType

NP16 = np.float16
NP8 = ml_dtypes.float8_e4m3fn

_CACHE = {}


# ----------------------------------------------------------------------------
# Host-side graph preprocessing (index manipulation + sharding only)
# ----------------------------------------------------------------------------

def _wrap_idxs(idx):
    """[n] int -> [128, n//16] int16 wrapped layout for dma_gather."""
    n = idx.shape[0]
    assert n % 16 == 0
    block = idx.reshape(n // 16, 16).T.astype(np.int16)
    return np.tile(block, (8, 1))


def _prep(x, edge_attr, edge_index):
    row = np.asarray(edge_index[0]).astype(np.int64)
    col = np.asarray(edge_index[1]).astype(np.int64)
    ea = np.asarray(edge_attr).reshape(-1).astype(np.float32)
    xf = np.asarray(x).reshape(-1).astype(np.float32)

    cnt_full = np.bincount(row, minlength=N).astype(np.float32)
    core_of = row // NB

    cores = []
    ebc_max = 0
    for k in range(NCORES):
        eids = np.nonzero(core_of == k)[0]
        order = np.argsort(row[eids], kind="stable")
        eids = eids[order]
        blk = (row[eids] - k * NB) // 128
        bc = np.bincount(blk, minlength=BLOCKS)
        ebc_max = max(ebc_max, int(bc.max()))
        cores.append((eids, blk, bc))

    Tb = max(2, (ebc_max + 127) // 128)
    EB = Tb * 128
    Epad = BLOCKS * EB
    ET = Epad // 128  # number of 128-edge tiles
    NCH = Epad // 512  # always integer: Epad = 20*Tb*128

    def trow(n):
        return (n // NB) * NPAD + (n % NB)

    per_core = []
    for k in range(NCORES):
        eids, blk, bc = cores[k]
        r = row[eids]
        c = col[eids]
        starts = np.zeros(BLOCKS, dtype=np.int64)
        np.cumsum(bc[:-1], out=starts[1:])
        pos_in_blk = np.arange(len(eids)) - starts[blk]
        dst = blk * EB + pos_in_blk

        gp = np.zeros(Epad, dtype=np.int64)
        gq = np.zeros(Epad, dtype=np.int64)
        slot = np.full(Epad, -1, dtype=np.int64)
        ea_s = np.ones(Epad, dtype=np.float32)
        dm = np.zeros(Epad, dtype=np.float32)
        orig = np.full(Epad, -1, dtype=np.int64)

        gp[dst] = trow(r)
        gq[dst] = trow(c)
        slot[dst] = (r - k * NB) % 128
        ea_s[dst] = ea[eids]
        dm[dst] = (r == c).astype(np.float32)
        orig[dst] = eids

        sel = np.zeros((Epad, 128), dtype=NP8)
        valid = slot >= 0
        sel[np.nonzero(valid)[0], slot[valid]] = NP8(1.0)
        selT = np.ascontiguousarray(sel.T)  # [128 slots, Epad]

        own = cnt_full[k * NB:(k + 1) * NB]
        tmp = np.zeros(BLOCKS * 128, dtype=np.float32)
        tmp[:NB] = own
        cnt_nm = tmp.reshape(BLOCKS, 128).T.copy()
        ind = np.zeros((1, NPAD), dtype=np.float32)
        ind[0, :NB] = (own > 0).astype(np.float32)

        x_own = np.zeros(NPAD, dtype=np.float32)
        x_own[:NB] = xf[k * NB:(k + 1) * NB]

        em = lambda a: a.reshape(ET, 128).T.copy()  # edge-slot-major [128, ET]
        per_core.append(dict(
            ea_row=ea_s.reshape(1, Epad).astype(NP16),
            x_row=x_own.reshape(1, NPAD).astype(np.float32),

            ea_em=em(ea_s),
            dm_em=em(dm).astype(NP16),
            dmc_em=em((1.0 - dm) * (slot >= 0)).astype(NP16),
            gq_idx=_wrap_idxs(gq),
            sel=sel,
            selT=selT,
            cnt_nm=cnt_nm,
            ind_r=ind,
            orig=orig,
        ))
    return per_core, Tb


def _weights_inputs(inp):
    """Build the weight/bias input arrays (shared across cores)."""
    g = lambda name: np.asarray(inp[name], dtype=np.float32)
    w = {}
    col = lambda a: a.reshape(128, 1).astype(np.float32)

    w["encn_W0"] = g("encn_W0").reshape(1, L)
    w["encn_b0"] = col(g("encn_b0"))
    w["encn_W1h"] = g("encn_W1").astype(NP16)
    w["encn_b1"] = col(g("encn_b1"))
    w["ence_W0"] = g("ence_W0").reshape(1, L).astype(NP16)
    w["ence_b0"] = col(g("ence_b0"))
    w["ence_W1h"] = g("ence_W1").astype(NP16)
    w["ence_b1"] = col(g("ence_b1"))
    eW0, eb0, eW1, eb1 = g("eW0"), g("eb0"), g("eW1"), g("eb1")
    nW0, nb0, nW1, nb1 = g("nW0"), g("nb0"), g("nW1"), g("nb1")
    for s in range(S):
        w[f"eW0ab_{s}"] = np.concatenate(
            [eW0[s, :L, :], eW0[s, L:2 * L, :]], axis=1).astype(NP16)
        w[f"eW0c_{s}"] = eW0[s, 2 * L:, :].astype(NP16)
        w[f"eb0bc_{s}"] = np.tile(eb0[s].reshape(1, L), (128, 1)).astype(NP16)
        w[f"eW1h_{s}"] = eW1[s].astype(NP16)
        w[f"eb1_{s}"] = col(eb1[s])
    for s in range(S - 1):
        w[f"eW1f_{s}"] = eW1[s]
        w[f"eb1row_{s}"] = eb1[s].reshape(1, L)
        w[f"nW0a_{s}"] = nW0[s, :L, :]
        w[f"nW0bh_{s}"] = nW0[s, L:, :].astype(NP16)
        w[f"nb0_{s}"] = col(nb0[s])
        w[f"nW1h_{s}"] = nW1[s].astype(NP16)
        w[f"nb1_{s}"] = col(nb1[s])
    w["dec_W0h"] = g("dec_W0").astype(NP16)
    w["dec_b0"] = col(g("dec_b0"))
    w["dec_W1h"] = g("dec_W1").reshape(L, 1).astype(NP16)
    w["dec_b1"] = np.full((128, 1), float(np.asarray(inp["dec_b1"]).reshape(-1)[0]),
                          dtype=np.float32)
    w["ident"] = np.eye(128, dtype=np.float32)
    w["ident16"] = np.eye(128, dtype=NP16)
    return w


# ----------------------------------------------------------------------------
# Device program
# ----------------------------------------------------------------------------

def _build(nc, Tb, w_shapes):
    kb_blocks = int(os.environ.get("KB_BLOCKS", str(BLOCKS)))
    kb_no_sel = bool(int(os.environ.get("KB_NO_SEL", "0")))
    kb_no_tp = bool(int(os.environ.get("KB_NO_TP", "0")))
    kb_no_gather = bool(int(os.environ.get("KB_NO_GATHER", "0")))
    kb_no_cc = bool(int(os.environ.get("KB_NO_CC", "0")))
    kb_no_dec = bool(int(os.environ.get("KB_NO_DEC", "0")))
    EB = Tb * 128
    Epad = BLOCKS * EB
    ET = Epad // 128
    NCH = Epad // 512
    # gather groups (in tiles) per block
    groups = []
    t0 = 0
    while t0 < Tb:
        groups.append((t0, min(GT, Tb - t0)))
        t0 += GT

    din = {}

    def inp(name, shape, dtype):
        din[name] = nc.dram_tensor(name, shape, dtype, kind="ExternalInput")
        return din[name]

    inp("ea_row", [1, Epad], F16)
    inp("x_row", [1, NPAD], F32)
    inp("ea_em", [128, ET], F32)
    inp("dm_em", [128, ET], F16)
    inp("dmc_em", [128, ET], F16)
    inp("gq_idx", [128, Epad // 16], I16)
    inp("sel", [Epad, 128], F8)
    inp("selT", [128, Epad], F8)
    inp("cnt_nm", [128, BLOCKS], F32)
    inp("ind_r", [1, NPAD], F32)
    for name, arr_shape, np_dtype in w_shapes:
        inp(name, list(arr_shape), F16 if np_dtype == NP16 else F32)

    out_em = nc.dram_tensor("out_em", [128, ET], F32, kind="ExternalOutput")

    with tile.TileContext(nc) as tc, ExitStack() as ctx:
        P = lambda name, bufs, **kw: ctx.enter_context(
            tc.tile_pool(name=name, bufs=bufs, **kw))
        const = P("const", 1)
        big = P("big", 1)
        dram = P("dram", 1, space="DRAM")
        selp = P("selp", 3)
        selTring = P("selTring", 3)
        gath = P("gath", 6)
        hring = P("hring", 6)
        hem = P("hem", 8)
        tring = P("tring", 4)
        sgring = P("sgring", 3)
        rows = P("rows", 2)
        aggring = P("aggring", 2)
        pqring = P("pqring", 2)
        ps_r = P("ps_r", 2, space="PSUM")
        ps_le = P("ps_le", 2, space="PSUM")
        ps_g = P("ps_g", 1, space="PSUM")
        ps_m = P("ps_m", 1, space="PSUM")
        ps_t = P("ps_t", 2, space="PSUM")

        # ---- load constants / weights ----
        W = {}
        for name, arr_shape, np_dtype in w_shapes:
            t = const.tile(list(arr_shape), F16 if np_dtype == NP16 else F32,
                           name=f"w_{name}")
            nc.sync.dma_start(t[:], din[name][:])
            W[name] = t
        gq_idx = const.tile([128, Epad // 16], I16, name="gq_idx_s")
        nc.sync.dma_start(gq_idx[:], din["gq_idx"][:])
        cnt = const.tile([128, BLOCKS], F32, name="cnt_s")
        nc.sync.dma_start(cnt[:], din["cnt_nm"][:])
        ind = const.tile([1, NPAD], F32, name="ind_s")
        nc.sync.dma_start(ind[:], din["ind_r"][:])
        ea_em = const.tile([128, ET], F32, name="ea_em_s")
        nc.sync.dma_start(ea_em[:], din["ea_em"][:])
        dm_em = const.tile([128, ET], F16, name="dm_em_s")
        nc.sync.dma_start(dm_em[:], din["dm_em"][:])
        dmc_em = const.tile([128, ET], F16, name="dmc_em_s")
        nc.sync.dma_start(dmc_em[:], din["dmc_em"][:])

        zeros = const.tile([128, 512], F16, name="zeros")
        nc.vector.memset(zeros[:], 0.0)
        invc = const.tile([128, BLOCKS], F32, name="invc")
        nc.vector.tensor_scalar_max(invc[:], cnt[:], 1.0)
        nc.vector.reciprocal(invc[:], invc[:])

        # ---- persistent big tensors ----
        le = big.tile([128, Epad], F16, name="le")       # edge latent (feature-major)
        ln = big.tile([128, NPAD], F32, name="ln")       # own-node latent
        sst = big.tile([128, NPAD], F32, name="sst")     # scaled segsum(h)^T
        p_own = big.tile([128, NPAD], F16, name="p_own")  # [slot, f] per block
        dec_em = big.tile([128, ET], F32, name="dec_em")
        cmb = big.tile([128, ET], F32, name="cmb")

        if kb_no_sel or kb_blocks < BLOCKS:
            nc.vector.memset(sst[:], 0.0)
        q_own = [dram.tile([NPAD, 128], F16, name=f"q_own_{s}") for s in range(S)]
        q_full = [dram.tile([TROWS, 128], F16, name=f"q_full_{s}",
                            addr_space="Shared") for s in range(S)]

        def mlp_rows(src_row, nrows, hidden_W0, b0, W1h, b1, dst, j0=0):
            """dst[:, 512j:...] = W1h.T @ relu(W0 (x) row_j + b0) + b1."""
            for j in range(j0, nrows):
                r = rows.tile([1, 512], hidden_W0.dtype, tag="rowin")
                nc.sync.dma_start(r[:], src_row[:, 512 * j:512 * (j + 1)])
                ps = ps_r.tile([128, 512], F32, tag="r")
                nc.tensor.matmul(ps[:], hidden_W0[:], r[:],
                                 start=True, stop=True)
                h0 = hring.tile([128, 512], F16, tag="h")
                nc.scalar.activation(h0[:], ps[:], AF.Relu, bias=b0[:])
                ps2 = ps_le.tile([128, 512], F32, tag="le")
                nc.tensor.matmul(ps2[:], W1h[:], h0[:], start=True, stop=True)
                nc.scalar.activation(dst[:, 512 * j:512 * (j + 1)], ps2[:],
                                     AF.Identity, bias=b1[:])

        # ---- node encoder (edge encoder emitted after AllGather 0) ----
        mlp_rows(din["x_row"], NPAD // 512, W["encn_W0"], W["encn_b0"],
                 W["encn_W1h"], W["encn_b1"], ln)

        def pq_block(s, j):
            l16 = hem.tile([128, 128], F16, tag="hem")
            nc.scalar.activation(l16[:], ln[:, 128 * j:128 * (j + 1)], AF.Copy)
            ps = ps_t.tile([128, 256], F32, tag="tp")
            nc.tensor.matmul(ps[:], l16[:], W[f"eW0ab_{s}"][:],
                             start=True, stop=True)
            nc.vector.tensor_tensor(
                p_own[:, 128 * j:128 * (j + 1)], ps[:, 0:128],
                W[f"eb0bc_{s}"][:], op=OP.add)
            t = pqring.tile([128, 128], F16, tag="pqe")
            nc.scalar.activation(t[:], ps[:, 128:256], AF.Copy)
            nc.sync.dma_start(q_own[s][128 * j:128 * (j + 1), :], t[:])

        def pq_cc(s):
            if kb_no_cc:
                nc.sync.dma_start(q_full[s][0:NPAD, :], q_own[s][:])
            else:
                nc.gpsimd.collective_compute(
                    "AllGather", OP.bypass,
                    replica_groups=[list(range(NCORES))],
                    ins=[q_own[s].opt()],
                    outs=[q_full[s].opt()],
                )

        def node_chunk(s, j):
            """agg + node MLP for node column chunk j (512 cols)."""
            o = 512 * j
            a_ps = ps_t.tile([128, 512], F32, tag="tp")
            nc.tensor.matmul(a_ps[:], W[f"eW1f_{s}"][:], sst[:, o:o + 512],
                             start=True, stop=False)
            nc.tensor.matmul(a_ps[:], W[f"eb1row_{s}"][:],
                             ind[:, o:o + 512], start=False, stop=True)
            agg = aggring.tile([128, 512], F16, tag="agg16")
            nc.scalar.activation(agg[:], a_ps[:], AF.Copy)
            p_ps = ps_r.tile([128, 512], F32, tag="r")
            nc.tensor.matmul(p_ps[:], W[f"nW0a_{s}"][:], ln[:, o:o + 512],
                             start=True, stop=False)
            nc.tensor.matmul(p_ps[:], W[f"nW0bh_{s}"][:], agg[:],
                             start=False, stop=True)
            hn = hring.tile([128, 512], F16, tag="h")
            nc.scalar.activation(hn[:], p_ps[:], AF.Relu,
                                 bias=W[f"nb0_{s}"][:])
            l_ps = ps_le.tile([128, 512], F32, tag="le")
            nc.tensor.matmul(l_ps[:], W[f"nW1h_{s}"][:], hn[:],
                             start=True, stop=True)
            nc.scalar.activation(ln[:, o:o + 512], l_ps[:],
                                 AF.Identity, bias=W[f"nb1_{s}"][:])

        for j in range(BLOCKS):
            pq_block(0, j)
        pq_cc(0)

        enc_done = 0

        def enc_upto(c_end):
            nonlocal enc_done
            if c_end > enc_done:
                mlp_rows(din["ea_row"], c_end, W["ence_W0"], W["ence_b0"],
                         W["ence_W1h"], W["ence_b1"], le, j0=enc_done)
                enc_done = c_end

        g_ctr = 0
        for s in range(S):
            do_agg = s < S - 1
            eW0c = W[f"eW0c_{s}"]
            eW1h = W[f"eW1h_{s}"]
            eb1 = W[f"eb1_{s}"]
            for b in range(kb_blocks):
                if s == 0:
                    enc_upto(min(NCH, ((b + 3) * EB + 511) // 512))
                selT_b = selTring.tile([128, EB], F8, tag="selT")
                nc.sync.dma_start(selT_b[:], din["selT"][:, b * EB:(b + 1) * EB])
                pob = p_own[:, 128 * b:128 * (b + 1)]
                if do_agg and not kb_no_sel:
                    g_ps = ps_g.tile([128, 128], F32, tag="g")
                for (gt0, gnt) in groups:
                    # gather this group's Q rows
                    i0 = b * EB + gt0 * 128
                    ni = gnt * 128
                    gq_t = gath.tile([128, 1, GT * 128], F16, tag="gq")
                    if kb_no_gather:
                        nc.vector.memset(gq_t[:], 0.0)
                    else:
                        nc.gpsimd.dma_gather(
                            gq_t[:, :, :ni], q_full[s][:, :],
                            gq_idx[:, i0 // 16:(i0 + ni) // 16],
                            num_idxs=ni, num_idxs_reg=ni,
                            elem_size=128, elem_step=128, transpose=True,
                            queue_num=g_ctr % 4)
                        g_ctr += 1
                    if do_agg and not kb_no_sel:
                        sel_t = selp.tile([128, GT, 128], F8, tag="sel")
                        nc.sync.dma_start(
                            sel_t[:, :gnt, :],
                            din["sel"][i0:i0 + ni, :].rearrange(
                                "(t p) s -> p t s", p=128))
                    # chunks of <=512 within the group
                    co = 0
                    while co < ni:
                        cw = min(512, ni - co)
                        goff = i0 + co            # global edge-slot offset
                        boff = gt0 * 128 + co     # offset within the block
                        ps = ps_r.tile([128, 512], F32, tag="r")
                        nc.tensor.matmul(ps[:, :cw], pob,
                                         selT_b[:, boff:boff + cw],
                                         start=True, stop=False)
                        nc.tensor.matmul(ps[:, :cw], eW0c[:],
                                         le[:, goff:goff + cw],
                                         start=False, stop=True)
                        t2 = tring.tile([128, 512], F16, tag="t")
                        nc.vector.tensor_tensor(t2[:, :cw],
                                                gq_t[:, 0, co:co + cw],
                                                ps[:, :cw], op=OP.add)
                        h = hring.tile([128, 512], F16, tag="h")
                        nc.vector.tensor_tensor(h[:, :cw], t2[:, :cw],
                                                zeros[:, :cw], op=OP.max)
                        ps2 = ps_le.tile([128, 512], F32, tag="le")
                        nc.tensor.matmul(ps2[:, :cw], eW1h[:], h[:, :cw],
                                         start=True, stop=True)
                        nc.scalar.activation(le[:, goff:goff + cw], ps2[:, :cw],
                                             AF.Identity, bias=eb1[:])
                        if do_agg and not kb_no_sel:
                            for u in range(cw // 128):
                                tt = gt0 + (co // 128) + u
                                he = hem.tile([128, 128], F16, tag="hem")
                                if kb_no_tp:
                                    nc.vector.tensor_copy(
                                        he[:], h[:, 128 * u:128 * (u + 1)])
                                else:
                                    ht_ps = ps_t.tile([128, 128], F16, tag="tp")
                                    nc.tensor.transpose(
                                        ht_ps[:], h[:, 128 * u:128 * (u + 1)],
                                        W["ident16"][:])
                                    if tt % 2 == 0:
                                        nc.scalar.activation(he[:], ht_ps[:],
                                                             AF.Copy)
                                    else:
                                        nc.vector.tensor_tensor(
                                            he[:], ht_ps[:], zeros[:, 0:128],
                                            op=OP.add)
                                nc.tensor.matmul(
                                    g_ps[:], sel_t[:, (co // 128) + u, :], he[:],
                                    start=(tt == 0), stop=(tt == Tb - 1),
                                    skip_group_check=True)
                        co += cw
                if do_agg and not kb_no_sel:
                    sg = sgring.tile([128, 128], F32, tag="sg")
                    nc.vector.tensor_tensor(sg[:], g_ps[:],
                                            invc[:, b:b + 1].to_broadcast(
                                                [128, 128]), op=OP.mult)
                    ps_tr = ps_m.tile([128, 128], F32, tag="m")
                    nc.tensor.transpose(ps_tr[:], sg[:], W["ident"][:])
                    nc.vector.tensor_tensor(sst[:, 128 * b:128 * (b + 1)],
                                            ps_tr[:], zeros[:, 0:128], op=OP.add)
                    if b % 4 == 3 and kb_blocks == BLOCKS:
                        node_chunk(s, b // 4)
                        for jb in range(4 * (b // 4), 4 * (b // 4) + 4):
                            pq_block(s + 1, jb)

            if do_agg:
                pq_cc(s + 1)

        # ---- decoder ----
        for ci in range(0 if kb_no_dec else NCH):
            off = 512 * ci
            ps = ps_r.tile([128, 512], F32, tag="r")
            nc.tensor.matmul(ps[:], W["dec_W0h"][:], le[:, off:off + 512],
                             start=True, stop=True)
            hd = hring.tile([128, 512], F16, tag="h")
            nc.scalar.activation(hd[:], ps[:], AF.Relu, bias=W["dec_b0"][:])
            d_ps = ps_m.tile([128, 4], F32, tag="m")
            for u in range(4):
                nc.tensor.matmul(d_ps[:, u:u + 1], hd[:, 128 * u:128 * (u + 1)],
                                 W["dec_W1h"][:], start=True, stop=True)
            nc.vector.tensor_tensor(dec_em[:, 4 * ci:4 * ci + 4], d_ps[:],
                                    W["dec_b1"][:].to_broadcast([128, 4]),
                                    op=OP.add)

        # ---- final combine: out = dm*0.5*sqrt(ea) + dmc*dec ----
        nc.scalar.sqrt(cmb[:], ea_em[:])
        nc.vector.scalar_tensor_tensor(cmb[:], dm_em[:], 0.5, cmb[:],
                                       op0=OP.mult, op1=OP.mult)
        nc.vector.tensor_tensor(dec_em[:], dmc_em[:], dec_em[:], op=OP.mult)
        nc.vector.tensor_tensor(cmb[:], cmb[:], dec_em[:], op=OP.add)
        nc.sync.dma_start(out_em[:], cmb[:])

    nc.compile()


# ----------------------------------------------------------------------------
# Entry point
# ----------------------------------------------------------------------------

def _get_program(Tb, w_shapes):
    key = Tb
    if key not in _CACHE:
        import time
        t0 = time.time()
        nc = bacc.Bacc("TRN2", target_bir_lowering=False, debug=False,
                       num_devices=NCORES, num_swdge_queues=4)
        _build(nc, Tb, w_shapes)
        if os.environ.get("KERNEL_VERBOSE"):
            print(f"[kernel] build+schedule+compile: {time.time()-t0:.1f}s",
                  flush=True)
        _CACHE[key] = nc
    return _CACHE[key]


def kernel(**inputs):
    per_core, Tb = _prep(inputs["x"], inputs["edge_attr"], inputs["edge_index"])
    w = _weights_inputs(inputs)
    w_shapes = [(k, v.shape, v.dtype.type) for k, v in w.items()]
    nc = _get_program(Tb, w_shapes)

    in_maps = []
    for k in range(NCORES):
        m = dict(w)
        pc = per_core[k]
        for key in ("ea_row", "x_row", "ea_em", "dm_em", "dmc_em", "gq_idx",
                    "sel", "selT", "cnt_nm", "ind_r"):
            m[key] = pc[key]
        in_maps.append(m)

    trace = bool(int(os.environ.get("KERNEL_TRACE", "0")))
    import time as _time
    _t0 = _time.time()
    res = run_bass_kernel_spmd(
        nc, in_maps, core_ids=list(range(NCORES)), trace=trace,
        tmpdir=os.environ.get("KERNEL_TRACE_DIR") or None)
    if os.environ.get("KERNEL_VERBOSE"):
        print(f"[kernel] exec phase: {_time.time()-_t0:.1f}s", flush=True)
    if trace:
        print(f"HW exec time: {res.exec_time_ns} ns")
        if res.instructions_and_trace:
            print("trace:", res.instructions_and_trace[1])

    out = np.zeros((E, 1), dtype=np.float32)
    ET = (BLOCKS * Tb * 128) // 128
    for k in range(NCORES):
        o = res.results[k]["out_em"]           # [128, ET]
        flat = o.T.reshape(-1)                 # slot order
        orig = per_core[k]["orig"]
        valid = orig >= 0
        out[orig[valid], 0] = flat[valid]
    return out



# revision 36
# speedup vs baseline: 1.7532x; 1.0555x over previous
"""Trainium2 Bass kernel for NeuralPCG GNN message passing (8 NeuronCores).

Strategy: destination-sharded edges (core k owns all edges whose dest node is
in its 2500-node range), feature-major fp16 matmuls, dma_gather for P/Q lookups,
one-hot SEL matmuls for segment sums, one AllGather of the fused P|Q table per
message-passing step.
"""
import os
import numpy as np
import ml_dtypes
from contextlib import ExitStack

import concourse.bass as bass
import concourse.tile as tile
from concourse import bacc, mybir
from concourse.bass_utils import run_bass_kernel_spmd

N = 20000
E = 320000
L = 128
S = 3
NCORES = 8
NB = 2500            # nodes per core
BLOCKS = 20          # 128-node blocks per core
NPAD = BLOCKS * 128  # 2560
TROWS = NCORES * NPAD  # 20480 rows in the AllGathered PQ table
GT = 6               # tiles per gather group (6*128 = 768 idxs;
                     # dma_gather with num_idxs=1024 hangs the device)

F32 = mybir.dt.float32
F16 = mybir.dt.float16
F8 = mybir.dt.float8e4
I16 = mybir.dt.int16
AF = mybir.ActivationFunctionType
OP = mybir.AluOpType

NP16 = np.float16
NP8 = ml_dtypes.float8_e4m3fn

_CACHE = {}


# ----------------------------------------------------------------------------
# Host-side graph preprocessing (index manipulation + sharding only)
# ----------------------------------------------------------------------------

def _wrap_idxs(idx):
    """[n] int -> [128, n//16] int16 wrapped layout for dma_gather."""
    n = idx.shape[0]
    assert n % 16 == 0
    block = idx.reshape(n // 16, 16).T.astype(np.int16)
    return np.tile(block, (8, 1))


def _prep(x, edge_attr, edge_index):
    row = np.asarray(edge_index[0]).astype(np.int64)
    col = np.asarray(edge_index[1]).astype(np.int64)
    ea = np.asarray(edge_attr).reshape(-1).astype(np.float32)
    xf = np.asarray(x).reshape(-1).astype(np.float32)

    cnt_full = np.bincount(row, minlength=N).astype(np.float32)
    core_of = row // NB

    cores = []
    ebc_max = 0
    for k in range(NCORES):
        eids = np.nonzero(core_of == k)[0]
        order = np.argsort(row[eids], kind="stable")
        eids = eids[order]
        blk = (row[eids] - k * NB) // 128
        bc = np.bincount(blk, minlength=BLOCKS)
        ebc_max = max(ebc_max, int(bc.max()))
        cores.append((eids, blk, bc))

    Tb = max(2, (ebc_max + 127) // 128)
    EB = Tb * 128
    Epad = BLOCKS * EB
    ET = Epad // 128  # number of 128-edge tiles
    NCH = Epad // 512  # always integer: Epad = 20*Tb*128

    def trow(n):
        return (n // NB) * NPAD + (n % NB)

    per_core = []
    for k in range(NCORES):
        eids, blk, bc = cores[k]
        r = row[eids]
        c = col[eids]
        starts = np.zeros(BLOCKS, dtype=np.int64)
        np.cumsum(bc[:-1], out=starts[1:])
        pos_in_blk = np.arange(len(eids)) - starts[blk]
        dst = blk * EB + pos_in_blk

        gp = np.zeros(Epad, dtype=np.int64)
        gq = np.zeros(Epad, dtype=np.int64)
        slot = np.full(Epad, -1, dtype=np.int64)
        ea_s = np.ones(Epad, dtype=np.float32)
        dm = np.zeros(Epad, dtype=np.float32)
        orig = np.full(Epad, -1, dtype=np.int64)

        gp[dst] = trow(r)
        gq[dst] = trow(c)
        slot[dst] = (r - k * NB) % 128
        ea_s[dst] = ea[eids]
        dm[dst] = (r == c).astype(np.float32)
        orig[dst] = eids

        sel = np.zeros((Epad, 128), dtype=NP8)
        valid = slot >= 0
        sel[np.nonzero(valid)[0], slot[valid]] = NP8(1.0)
        selT = np.ascontiguousarray(sel.T)  # [128 slots, Epad]

        own = cnt_full[k * NB:(k + 1) * NB]
        tmp = np.zeros(BLOCKS * 128, dtype=np.float32)
        tmp[:NB] = own
        cnt_nm = tmp.reshape(BLOCKS, 128).T.copy()
        ind = np.zeros((1, NPAD), dtype=np.float32)
        ind[0, :NB] = (own > 0).astype(np.float32)

        x_own = np.zeros(NPAD, dtype=np.float32)
        x_own[:NB] = xf[k * NB:(k + 1) * NB]

        em = lambda a: a.reshape(ET, 128).T.copy()  # edge-slot-major [128, ET]
        per_core.append(dict(
            ea_row=ea_s.reshape(1, Epad).astype(NP16),
            x_row=x_own.reshape(1, NPAD).astype(np.float32),

            ea_em=em(ea_s),
            dm_em=em(dm).astype(NP16),
            dmc_em=em((1.0 - dm) * (slot >= 0)).astype(NP16),
            gq_idx=_wrap_idxs(gq),
            sel=sel,
            selT=selT,
            cnt_nm=cnt_nm,
            ind_r=ind,
            orig=orig,
        ))
    return per_core, Tb


def _weights_inputs(inp):
    """Build the weight/bias input arrays (shared across cores)."""
    g = lambda name: np.asarray(inp[name], dtype=np.float32)
    w = {}
    col = lambda a: a.reshape(128, 1).astype(np.float32)

    w["encn_W0"] = g("encn_W0").reshape(1, L)
    w["encn_b0"] = col(g("encn_b0"))
    w["encn_W1h"] = g("encn_W1").astype(NP16)
    w["encn_b1"] = col(g("encn_b1"))
    w["ence_W0"] = g("ence_W0").reshape(1, L).astype(NP16)
    w["ence_b0"] = col(g("ence_b0"))
    w["ence_W1h"] = g("ence_W1").astype(NP16)
    w["ence_b1"] = col(g("ence_b1"))
    eW0, eb0, eW1, eb1 = g("eW0"), g("eb0"), g("eW1"), g("eb1")
    nW0, nb0, nW1, nb1 = g("nW0"), g("nb0"), g("nW1"), g("nb1")
    for s in range(S):
        w[f"eW0ab_{s}"] = np.concatenate(
            [eW0[s, :L, :], eW0[s, L:2 * L, :]], axis=1).astype(NP16)
        w[f"eW0c_{s}"] = eW0[s, 2 * L:, :].astype(NP16)
        w[f"eb0bc_{s}"] = np.tile(eb0[s].reshape(1, L), (128, 1)).astype(NP16)
        w[f"eW1h_{s}"] = eW1[s].astype(NP16)
        w[f"eb1_{s}"] = col(eb1[s])
    for s in range(S - 1):
        w[f"eW1f_{s}"] = eW1[s]
        w[f"eb1row_{s}"] = eb1[s].reshape(1, L)
        w[f"nW0a_{s}"] = nW0[s, :L, :]
        w[f"nW0bh_{s}"] = nW0[s, L:, :].astype(NP16)
        w[f"nb0_{s}"] = col(nb0[s])
        w[f"nW1h_{s}"] = nW1[s].astype(NP16)
        w[f"nb1_{s}"] = col(nb1[s])
    w["dec_W0h"] = g("dec_W0").astype(NP16)
    w["dec_b0"] = col(g("dec_b0"))
    w["dec_W1h"] = g("dec_W1").reshape(L, 1).astype(NP16)
    w["dec_b1"] = np.full((128, 1), float(np.asarray(inp["dec_b1"]).reshape(-1)[0]),
                          dtype=np.float32)
    w["ident"] = np.eye(128, dtype=np.float32)
    w["ident16"] = np.eye(128, dtype=NP16)
    return w


# ----------------------------------------------------------------------------
# Device program
# ----------------------------------------------------------------------------

def _build(nc, Tb, w_shapes):
    kb_blocks = int(os.environ.get("KB_BLOCKS", str(BLOCKS)))
    kb_no_sel = bool(int(os.environ.get("KB_NO_SEL", "0")))
    kb_no_tp = bool(int(os.environ.get("KB_NO_TP", "0")))
    kb_no_gather = bool(int(os.environ.get("KB_NO_GATHER", "0")))
    kb_no_cc = bool(int(os.environ.get("KB_NO_CC", "0")))
    kb_no_dec = bool(int(os.environ.get("KB_NO_DEC", "0")))
    EB = Tb * 128
    Epad = BLOCKS * EB
    ET = Epad // 128
    NCH = Epad // 512
    # gather groups (in tiles) per block
    groups = []
    t0 = 0
    while t0 < Tb:
        groups.append((t0, min(GT, Tb - t0)))
        t0 += GT

    din = {}

    def inp(name, shape, dtype):
        din[name] = nc.dram_tensor(name, shape, dtype, kind="ExternalInput")
        return din[name]

    inp("ea_row", [1, Epad], F16)
    inp("x_row", [1, NPAD], F32)
    inp("ea_em", [128, ET], F32)
    inp("dm_em", [128, ET], F16)
    inp("dmc_em", [128, ET], F16)
    inp("gq_idx", [128, Epad // 16], I16)
    inp("sel", [Epad, 128], F8)
    inp("selT", [128, Epad], F8)
    inp("cnt_nm", [128, BLOCKS], F32)
    inp("ind_r", [1, NPAD], F32)
    for name, arr_shape, np_dtype in w_shapes:
        inp(name, list(arr_shape), F16 if np_dtype == NP16 else F32)

    out_em = nc.dram_tensor("out_em", [128, ET], F32, kind="ExternalOutput")

    with tile.TileContext(nc) as tc, ExitStack() as ctx:
        P = lambda name, bufs, **kw: ctx.enter_context(
            tc.tile_pool(name=name, bufs=bufs, **kw))
        const = P("const", 1)
        big = P("big", 1)
        dram = P("dram", 1, space="DRAM")
        selp = P("selp", 3)
        selTring = P("selTring", 3)
        gath = P("gath", 6)
        hring = P("hring", 6)
        hem = P("hem", 8)
        tring = P("tring", 4)
        sgring = P("sgring", 3)
        rows = P("rows", 2)
        aggring = P("aggring", 2)
        pqring = P("pqring", 2)
        ps_r = P("ps_r", 2, space="PSUM")
        ps_le = P("ps_le", 2, space="PSUM")
        ps_g = P("ps_g", 1, space="PSUM")
        ps_m = P("ps_m", 1, space="PSUM")
        ps_t = P("ps_t", 2, space="PSUM")

        # ---- load constants / weights ----
        W = {}
        for name, arr_shape, np_dtype in w_shapes:
            t = const.tile(list(arr_shape), F16 if np_dtype == NP16 else F32,
                           name=f"w_{name}")
            nc.sync.dma_start(t[:], din[name][:])
            W[name] = t
        gq_idx = const.tile([128, Epad // 16], I16, name="gq_idx_s")
        nc.sync.dma_start(gq_idx[:], din["gq_idx"][:])
        cnt = const.tile([128, BLOCKS], F32, name="cnt_s")
        nc.sync.dma_start(cnt[:], din["cnt_nm"][:])
        ind = const.tile([1, NPAD], F32, name="ind_s")
        nc.sync.dma_start(ind[:], din["ind_r"][:])
        ea_em = const.tile([128, ET], F32, name="ea_em_s")
        nc.sync.dma_start(ea_em[:], din["ea_em"][:])
        dm_em = const.tile([128, ET], F16, name="dm_em_s")
        nc.sync.dma_start(dm_em[:], din["dm_em"][:])
        dmc_em = const.tile([128, ET], F16, name="dmc_em_s")
        nc.sync.dma_start(dmc_em[:], din["dmc_em"][:])

        zeros = const.tile([128, 512], F16, name="zeros")
        nc.vector.memset(zeros[:], 0.0)
        invc = const.tile([128, BLOCKS], F32, name="invc")
        nc.vector.tensor_scalar_max(invc[:], cnt[:], 1.0)
        nc.vector.reciprocal(invc[:], invc[:])

        # ---- persistent big tensors ----
        le = big.tile([128, Epad], F16, name="le")       # edge latent (feature-major)
        ln = big.tile([128, NPAD], F32, name="ln")       # own-node latent
        sst = big.tile([128, NPAD], F32, name="sst")     # scaled segsum(h)^T
        p_own = big.tile([128, NPAD], F16, name="p_own")  # [slot, f] per block
        dec_em = big.tile([128, ET], F32, name="dec_em")
        cmb = big.tile([128, ET], F32, name="cmb")

        if kb_no_sel or kb_blocks < BLOCKS:
            nc.vector.memset(sst[:], 0.0)
        q_own = [dram.tile([NPAD, 128], F16, name=f"q_own_{s}") for s in range(S)]
        q_full = [dram.tile([TROWS, 128], F16, name=f"q_full_{s}",
                            addr_space="Shared") for s in range(S)]

        def mlp_rows(src_row, nrows, hidden_W0, b0, W1h, b1, dst, j0=0):
            """dst[:, 512j:...] = W1h.T @ relu(W0 (x) row_j + b0) + b1."""
            for j in range(j0, nrows):
                r = rows.tile([1, 512], hidden_W0.dtype, tag="rowin")
                nc.sync.dma_start(r[:], src_row[:, 512 * j:512 * (j + 1)])
                ps = ps_r.tile([128, 512], F32, tag="r")
                nc.tensor.matmul(ps[:], hidden_W0[:], r[:],
                                 start=True, stop=True)
                h0 = hring.tile([128, 512], F16, tag="h")
                nc.scalar.activation(h0[:], ps[:], AF.Relu, bias=b0[:])
                ps2 = ps_le.tile([128, 512], F32, tag="le")
                nc.tensor.matmul(ps2[:], W1h[:], h0[:], start=True, stop=True)
                nc.scalar.activation(dst[:, 512 * j:512 * (j + 1)], ps2[:],
                                     AF.Identity, bias=b1[:])

        # ---- node encoder (edge encoder emitted after AllGather 0) ----
        mlp_rows(din["x_row"], NPAD // 512, W["encn_W0"], W["encn_b0"],
                 W["encn_W1h"], W["encn_b1"], ln)

        def pq_block(s, j):
            l16 = hem.tile([128, 128], F16, tag="hem")
            nc.scalar.activation(l16[:], ln[:, 128 * j:128 * (j + 1)], AF.Copy)
            ps = ps_t.tile([128, 256], F32, tag="tp")
            nc.tensor.matmul(ps[:], l16[:], W[f"eW0ab_{s}"][:],
                             start=True, stop=True)
            nc.vector.tensor_tensor(
                p_own[:, 128 * j:128 * (j + 1)], ps[:, 0:128],
                W[f"eb0bc_{s}"][:], op=OP.add)
            t = pqring.tile([128, 128], F16, tag="pqe")
            nc.scalar.activation(t[:], ps[:, 128:256], AF.Copy)
            nc.sync.dma_start(q_own[s][128 * j:128 * (j + 1), :], t[:])

        def pq_cc(s):
            if kb_no_cc:
                nc.sync.dma_start(q_full[s][0:NPAD, :], q_own[s][:])
            else:
                nc.gpsimd.collective_compute(
                    "AllGather", OP.bypass,
                    replica_groups=[list(range(NCORES))],
                    ins=[q_own[s].opt()],
                    outs=[q_full[s].opt()],
                )

        def node_chunk(s, j):
            """agg + node MLP for node column chunk j (512 cols)."""
            o = 512 * j
            a_ps = ps_t.tile([128, 512], F32, tag="tp")
            nc.tensor.matmul(a_ps[:], W[f"eW1f_{s}"][:], sst[:, o:o + 512],
                             start=True, stop=False)
            nc.tensor.matmul(a_ps[:], W[f"eb1row_{s}"][:],
                             ind[:, o:o + 512], start=False, stop=True)
            agg = aggring.tile([128, 512], F16, tag="agg16")
            nc.scalar.activation(agg[:], a_ps[:], AF.Copy)
            p_ps = ps_r.tile([128, 512], F32, tag="r")
            nc.tensor.matmul(p_ps[:], W[f"nW0a_{s}"][:], ln[:, o:o + 512],
                             start=True, stop=False)
            nc.tensor.matmul(p_ps[:], W[f"nW0bh_{s}"][:], agg[:],
                             start=False, stop=True)
            hn = hring.tile([128, 512], F16, tag="h")
            nc.scalar.activation(hn[:], p_ps[:], AF.Relu,
                                 bias=W[f"nb0_{s}"][:])
            l_ps = ps_le.tile([128, 512], F32, tag="le")
            nc.tensor.matmul(l_ps[:], W[f"nW1h_{s}"][:], hn[:],
                             start=True, stop=True)
            nc.scalar.activation(ln[:, o:o + 512], l_ps[:],
                                 AF.Identity, bias=W[f"nb1_{s}"][:])

        for j in range(BLOCKS):
            pq_block(0, j)
        pq_cc(0)

        dec_done = 0

        def dec_chunk(ci):
            off = 512 * ci
            ps = ps_r.tile([128, 512], F32, tag="r")
            nc.tensor.matmul(ps[:], W["dec_W0h"][:], le[:, off:off + 512],
                             start=True, stop=True)
            hd = hring.tile([128, 512], F16, tag="h")
            nc.scalar.activation(hd[:], ps[:], AF.Relu, bias=W["dec_b0"][:])
            d_ps = ps_m.tile([128, 4], F32, tag="m")
            for u in range(4):
                nc.tensor.matmul(d_ps[:, u:u + 1], hd[:, 128 * u:128 * (u + 1)],
                                 W["dec_W1h"][:], start=True, stop=True)
            nc.vector.tensor_tensor(dec_em[:, 4 * ci:4 * ci + 4], d_ps[:],
                                    W["dec_b1"][:].to_broadcast([128, 4]),
                                    op=OP.add)

        def dec_upto(c_end):
            nonlocal dec_done
            while dec_done < c_end:
                dec_chunk(dec_done)
                dec_done += 1

        enc_done = 0

        def enc_upto(c_end):
            nonlocal enc_done
            if c_end > enc_done:
                mlp_rows(din["ea_row"], c_end, W["ence_W0"], W["ence_b0"],
                         W["ence_W1h"], W["ence_b1"], le, j0=enc_done)
                enc_done = c_end

        g_ctr = 0
        for s in range(S):
            do_agg = s < S - 1
            eW0c = W[f"eW0c_{s}"]
            eW1h = W[f"eW1h_{s}"]
            eb1 = W[f"eb1_{s}"]
            for b in range(kb_blocks):
                if s == 0:
                    enc_upto(min(NCH, ((b + 3) * EB + 511) // 512))
                selT_b = selTring.tile([128, EB], F8, tag="selT")
                nc.sync.dma_start(selT_b[:], din["selT"][:, b * EB:(b + 1) * EB])
                pob = p_own[:, 128 * b:128 * (b + 1)]
                if do_agg and not kb_no_sel:
                    g_ps = ps_g.tile([128, 128], F32, tag="g")
                for (gt0, gnt) in groups:
                    # gather this group's Q rows
                    i0 = b * EB + gt0 * 128
                    ni = gnt * 128
                    gq_t = gath.tile([128, 1, GT * 128], F16, tag="gq")
                    if kb_no_gather:
                        nc.vector.memset(gq_t[:], 0.0)
                    else:
                        nc.gpsimd.dma_gather(
                            gq_t[:, :, :ni], q_full[s][:, :],
                            gq_idx[:, i0 // 16:(i0 + ni) // 16],
                            num_idxs=ni, num_idxs_reg=ni,
                            elem_size=128, elem_step=128, transpose=True,
                            queue_num=g_ctr % 4)
                        g_ctr += 1
                    if do_agg and not kb_no_sel:
                        sel_t = selp.tile([128, GT, 128], F8, tag="sel")
                        nc.sync.dma_start(
                            sel_t[:, :gnt, :],
                            din["sel"][i0:i0 + ni, :].rearrange(
                                "(t p) s -> p t s", p=128))
                    # chunks of <=512 within the group
                    co = 0
                    while co < ni:
                        cw = min(512, ni - co)
                        goff = i0 + co            # global edge-slot offset
                        boff = gt0 * 128 + co     # offset within the block
                        ps = ps_r.tile([128, 512], F32, tag="r")
                        nc.tensor.matmul(ps[:, :cw], pob,
                                         selT_b[:, boff:boff + cw],
                                         start=True, stop=False)
                        nc.tensor.matmul(ps[:, :cw], eW0c[:],
                                         le[:, goff:goff + cw],
                                         start=False, stop=True)
                        t2 = tring.tile([128, 512], F16, tag="t")
                        nc.vector.tensor_tensor(t2[:, :cw],
                                                gq_t[:, 0, co:co + cw],
                                                ps[:, :cw], op=OP.add)
                        h = hring.tile([128, 512], F16, tag="h")
                        nc.vector.tensor_tensor(h[:, :cw], t2[:, :cw],
                                                zeros[:, :cw], op=OP.max)
                        ps2 = ps_le.tile([128, 512], F32, tag="le")
                        nc.tensor.matmul(ps2[:, :cw], eW1h[:], h[:, :cw],
                                         start=True, stop=True)
                        nc.scalar.activation(le[:, goff:goff + cw], ps2[:, :cw],
                                             AF.Identity, bias=eb1[:])
                        if do_agg and not kb_no_sel:
                            for u in range(cw // 128):
                                tt = gt0 + (co // 128) + u
                                he = hem.tile([128, 128], F16, tag="hem")
                                if kb_no_tp:
                                    nc.vector.tensor_copy(
                                        he[:], h[:, 128 * u:128 * (u + 1)])
                                else:
                                    ht_ps = ps_t.tile([128, 128], F16, tag="tp")
                                    nc.tensor.transpose(
                                        ht_ps[:], h[:, 128 * u:128 * (u + 1)],
                                        W["ident16"][:])
                                    if tt % 2 == 0:
                                        nc.scalar.activation(he[:], ht_ps[:],
                                                             AF.Copy)
                                    else:
                                        nc.vector.tensor_tensor(
                                            he[:], ht_ps[:], zeros[:, 0:128],
                                            op=OP.add)
                                nc.tensor.matmul(
                                    g_ps[:], sel_t[:, (co // 128) + u, :], he[:],
                                    start=(tt == 0), stop=(tt == Tb - 1),
                                    skip_group_check=True)
                        co += cw
                if s == S - 1 and not kb_no_dec:
                    dec_upto(((b + 1) * EB) // 512)
                if do_agg and not kb_no_sel:
                    sg = sgring.tile([128, 128], F32, tag="sg")
                    nc.vector.tensor_tensor(sg[:], g_ps[:],
                                            invc[:, b:b + 1].to_broadcast(
                                                [128, 128]), op=OP.mult)
                    ps_tr = ps_m.tile([128, 128], F32, tag="m")
                    nc.tensor.transpose(ps_tr[:], sg[:], W["ident"][:])
                    nc.vector.tensor_tensor(sst[:, 128 * b:128 * (b + 1)],
                                            ps_tr[:], zeros[:, 0:128], op=OP.add)
                    if b % 4 == 3 and kb_blocks == BLOCKS:
                        node_chunk(s, b // 4)
                        for jb in range(4 * (b // 4), 4 * (b // 4) + 4):
                            pq_block(s + 1, jb)

            if do_agg:
                pq_cc(s + 1)

        # ---- decoder leftovers (most chunks interleaved into step 2) ----
        if not kb_no_dec:
            dec_upto(NCH)

        # ---- final combine: out = dm*0.5*sqrt(ea) + dmc*dec ----
        nc.scalar.sqrt(cmb[:], ea_em[:])
        nc.vector.scalar_tensor_tensor(cmb[:], dm_em[:], 0.5, cmb[:],
                                       op0=OP.mult, op1=OP.mult)
        nc.vector.tensor_tensor(dec_em[:], dmc_em[:], dec_em[:], op=OP.mult)
        nc.vector.tensor_tensor(cmb[:], cmb[:], dec_em[:], op=OP.add)
        nc.sync.dma_start(out_em[:], cmb[:])

    nc.compile()


# ----------------------------------------------------------------------------
# Entry point
# ----------------------------------------------------------------------------

def _get_program(Tb, w_shapes):
    key = Tb
    if key not in _CACHE:
        import time
        t0 = time.time()
        nc = bacc.Bacc("TRN2", target_bir_lowering=False, debug=False,
                       num_devices=NCORES, num_swdge_queues=4)
        _build(nc, Tb, w_shapes)
        if os.environ.get("KERNEL_VERBOSE"):
            print(f"[kernel] build+schedule+compile: {time.time()-t0:.1f}s",
                  flush=True)
        _CACHE[key] = nc
    return _CACHE[key]


def kernel(**inputs):
    per_core, Tb = _prep(inputs["x"], inputs["edge_attr"], inputs["edge_index"])
    w = _weights_inputs(inputs)
    w_shapes = [(k, v.shape, v.dtype.type) for k, v in w.items()]
    nc = _get_program(Tb, w_shapes)

    in_maps = []
    for k in range(NCORES):
        m = dict(w)
        pc = per_core[k]
        for key in ("ea_row", "x_row", "ea_em", "dm_em", "dmc_em", "gq_idx",
                    "sel", "selT", "cnt_nm", "ind_r"):
            m[key] = pc[key]
        in_maps.append(m)

    trace = bool(int(os.environ.get("KERNEL_TRACE", "0")))
    import time as _time
    _t0 = _time.time()
    res = run_bass_kernel_spmd(
        nc, in_maps, core_ids=list(range(NCORES)), trace=trace,
        tmpdir=os.environ.get("KERNEL_TRACE_DIR") or None)
    if os.environ.get("KERNEL_VERBOSE"):
        print(f"[kernel] exec phase: {_time.time()-_t0:.1f}s", flush=True)
    if trace:
        print(f"HW exec time: {res.exec_time_ns} ns")
        if res.instructions_and_trace:
            print("trace:", res.instructions_and_trace[1])

    out = np.zeros((E, 1), dtype=np.float32)
    ET = (BLOCKS * Tb * 128) // 128
    for k in range(NCORES):
        o = res.results[k]["out_em"]           # [128, ET]
        flat = o.T.reshape(-1)                 # slot order
        orig = per_core[k]["orig"]
        valid = orig >= 0
        out[orig[valid], 0] = flat[valid]
    return out



# revision 38
# speedup vs baseline: 1.7875x; 1.0195x over previous
"""Trainium2 Bass kernel for NeuralPCG GNN message passing (8 NeuronCores).

Strategy: destination-sharded edges (core k owns all edges whose dest node is
in its 2500-node range), feature-major fp16 matmuls. Per message-passing step:
the P-side (ln[row] @ eW0a) is expanded locally from SBUF-resident per-block
projections via one-hot selT matmuls (rows are core-local by construction);
the Q-side (ln[col] @ eW0b) is AllGathered (Shared output) into a 20480-row
fp16 table and fetched per edge with dma_gather spread across the 4 SWDGE
queues. Segment sums use one-hot sel matmuls on PE-transposed hidden tiles.

Perf-critical invariants learned on HW:
 - dma_gather desc-gen parallelizes ~3x across the 4 SWDGE queues.
 - DVE tensor_scalar/copy/cast in 2-port perf mode block GpSimd's shared
   SBUF port and stall gather desc-gen; hot-path DVE ops must be
   tensor_tensor (2x_1P) only.
 - One-hot sel/selT matrices stream as fp8e4 (exact for 0/1), halving
   their HBM traffic; mixed fp8 x fp16 matmuls are fine.
 - node-MLP / next-step projections / edge-encoder / decoder are all
   interleaved into the edge-block loop so collectives and tails overlap.
"""
import os
import numpy as np
import ml_dtypes
from contextlib import ExitStack

import concourse.bass as bass
import concourse.tile as tile
from concourse import bacc, mybir
from concourse.bass_utils import run_bass_kernel_spmd

N = 20000
E = 320000
L = 128
S = 3
NCORES = 8
NB = 2500            # nodes per core
BLOCKS = 20          # 128-node blocks per core
NPAD = BLOCKS * 128  # 2560
TROWS = NCORES * NPAD  # 20480 rows in the AllGathered PQ table
GT = 6               # tiles per gather group (6*128 = 768 idxs;
                     # dma_gather with num_idxs=1024 hangs the device)

F32 = mybir.dt.float32
F16 = mybir.dt.float16
F8 = mybir.dt.float8e4
I16 = mybir.dt.int16
AF = mybir.ActivationFunctionType
OP = mybir.AluOpType

NP16 = np.float16
NP8 = ml_dtypes.float8_e4m3fn

_CACHE = {}


# ----------------------------------------------------------------------------
# Host-side graph preprocessing (index manipulation + sharding only)
# ----------------------------------------------------------------------------

def _wrap_idxs(idx):
    """[n] int -> [128, n//16] int16 wrapped layout for dma_gather."""
    n = idx.shape[0]
    assert n % 16 == 0
    block = idx.reshape(n // 16, 16).T.astype(np.int16)
    return np.tile(block, (8, 1))


def _prep(x, edge_attr, edge_index):
    row = np.asarray(edge_index[0]).astype(np.int64)
    col = np.asarray(edge_index[1]).astype(np.int64)
    ea = np.asarray(edge_attr).reshape(-1).astype(np.float32)
    xf = np.asarray(x).reshape(-1).astype(np.float32)

    cnt_full = np.bincount(row, minlength=N).astype(np.float32)
    core_of = row // NB

    cores = []
    ebc_max = 0
    for k in range(NCORES):
        eids = np.nonzero(core_of == k)[0]
        order = np.argsort(row[eids], kind="stable")
        eids = eids[order]
        blk = (row[eids] - k * NB) // 128
        bc = np.bincount(blk, minlength=BLOCKS)
        ebc_max = max(ebc_max, int(bc.max()))
        cores.append((eids, blk, bc))

    Tb = max(2, (ebc_max + 127) // 128)
    EB = Tb * 128
    Epad = BLOCKS * EB
    ET = Epad // 128  # number of 128-edge tiles
    NCH = Epad // 512  # always integer: Epad = 20*Tb*128

    def trow(n):
        return (n // NB) * NPAD + (n % NB)

    per_core = []
    for k in range(NCORES):
        eids, blk, bc = cores[k]
        r = row[eids]
        c = col[eids]
        starts = np.zeros(BLOCKS, dtype=np.int64)
        np.cumsum(bc[:-1], out=starts[1:])
        pos_in_blk = np.arange(len(eids)) - starts[blk]
        dst = blk * EB + pos_in_blk

        gp = np.zeros(Epad, dtype=np.int64)
        gq = np.zeros(Epad, dtype=np.int64)
        slot = np.full(Epad, -1, dtype=np.int64)
        ea_s = np.ones(Epad, dtype=np.float32)
        dm = np.zeros(Epad, dtype=np.float32)
        orig = np.full(Epad, -1, dtype=np.int64)

        gp[dst] = trow(r)
        gq[dst] = trow(c)
        slot[dst] = (r - k * NB) % 128
        ea_s[dst] = ea[eids]
        dm[dst] = (r == c).astype(np.float32)
        orig[dst] = eids

        sel = np.zeros((Epad, 128), dtype=NP8)
        valid = slot >= 0
        sel[np.nonzero(valid)[0], slot[valid]] = NP8(1.0)
        selT = np.ascontiguousarray(sel.T)  # [128 slots, Epad]

        own = cnt_full[k * NB:(k + 1) * NB]
        tmp = np.zeros(BLOCKS * 128, dtype=np.float32)
        tmp[:NB] = own
        cnt_nm = tmp.reshape(BLOCKS, 128).T.copy()
        ind = np.zeros((1, NPAD), dtype=np.float32)
        ind[0, :NB] = (own > 0).astype(np.float32)

        x_own = np.zeros(NPAD, dtype=np.float32)
        x_own[:NB] = xf[k * NB:(k + 1) * NB]

        em = lambda a: a.reshape(ET, 128).T.copy()  # edge-slot-major [128, ET]
        per_core.append(dict(
            ea_row=ea_s.reshape(1, Epad).astype(NP16),
            x_row=x_own.reshape(1, NPAD).astype(np.float32),

            ea_em=em(ea_s),
            dm_em=em(dm).astype(NP16),
            dmc_em=em((1.0 - dm) * (slot >= 0)).astype(NP16),
            gq_idx=_wrap_idxs(gq),
            sel=sel,
            selT=selT,
            cnt_nm=cnt_nm,
            ind_r=ind,
            orig=orig,
        ))
    return per_core, Tb


def _weights_inputs(inp):
    """Build the weight/bias input arrays (shared across cores)."""
    g = lambda name: np.asarray(inp[name], dtype=np.float32)
    w = {}
    col = lambda a: a.reshape(128, 1).astype(np.float32)

    w["encn_W0"] = g("encn_W0").reshape(1, L)
    w["encn_b0"] = col(g("encn_b0"))
    w["encn_W1h"] = g("encn_W1").astype(NP16)
    w["encn_b1"] = col(g("encn_b1"))
    w["ence_W0"] = g("ence_W0").reshape(1, L).astype(NP16)
    w["ence_b0"] = col(g("ence_b0"))
    w["ence_W1h"] = g("ence_W1").astype(NP16)
    w["ence_b1"] = col(g("ence_b1"))
    eW0, eb0, eW1, eb1 = g("eW0"), g("eb0"), g("eW1"), g("eb1")
    nW0, nb0, nW1, nb1 = g("nW0"), g("nb0"), g("nW1"), g("nb1")
    for s in range(S):
        w[f"eW0ab_{s}"] = np.concatenate(
            [eW0[s, :L, :], eW0[s, L:2 * L, :]], axis=1).astype(NP16)
        w[f"eW0c_{s}"] = eW0[s, 2 * L:, :].astype(NP16)
        w[f"eb0bc_{s}"] = np.tile(eb0[s].reshape(1, L), (128, 1)).astype(NP16)
        w[f"eW1h_{s}"] = eW1[s].astype(NP16)
        w[f"eb1_{s}"] = col(eb1[s])
    for s in range(S - 1):
        w[f"eW1f_{s}"] = eW1[s]
        w[f"eb1row_{s}"] = eb1[s].reshape(1, L)
        w[f"nW0a_{s}"] = nW0[s, :L, :]
        w[f"nW0bh_{s}"] = nW0[s, L:, :].astype(NP16)
        w[f"nb0_{s}"] = col(nb0[s])
        w[f"nW1h_{s}"] = nW1[s].astype(NP16)
        w[f"nb1_{s}"] = col(nb1[s])
    w["dec_W0h"] = g("dec_W0").astype(NP16)
    w["dec_b0"] = col(g("dec_b0"))
    w["dec_W1h"] = g("dec_W1").reshape(L, 1).astype(NP16)
    w["dec_b1"] = np.full((128, 1), float(np.asarray(inp["dec_b1"]).reshape(-1)[0]),
                          dtype=np.float32)
    w["ident"] = np.eye(128, dtype=np.float32)
    w["ident16"] = np.eye(128, dtype=NP16)
    return w


# ----------------------------------------------------------------------------
# Device program
# ----------------------------------------------------------------------------

def _build(nc, Tb, w_shapes):
    kb_blocks = int(os.environ.get("KB_BLOCKS", str(BLOCKS)))
    kb_no_sel = bool(int(os.environ.get("KB_NO_SEL", "0")))
    kb_no_tp = bool(int(os.environ.get("KB_NO_TP", "0")))
    kb_no_gather = bool(int(os.environ.get("KB_NO_GATHER", "0")))
    kb_no_cc = bool(int(os.environ.get("KB_NO_CC", "0")))
    kb_no_dec = bool(int(os.environ.get("KB_NO_DEC", "0")))
    EB = Tb * 128
    Epad = BLOCKS * EB
    ET = Epad // 128
    NCH = Epad // 512
    # gather groups (in tiles) per block
    groups = []
    t0 = 0
    while t0 < Tb:
        groups.append((t0, min(GT, Tb - t0)))
        t0 += GT

    din = {}

    def inp(name, shape, dtype):
        din[name] = nc.dram_tensor(name, shape, dtype, kind="ExternalInput")
        return din[name]

    inp("ea_row", [1, Epad], F16)
    inp("x_row", [1, NPAD], F32)
    inp("ea_em", [128, ET], F32)
    inp("dm_em", [128, ET], F16)
    inp("dmc_em", [128, ET], F16)
    inp("gq_idx", [128, Epad // 16], I16)
    inp("sel", [Epad, 128], F8)
    inp("selT", [128, Epad], F8)
    inp("cnt_nm", [128, BLOCKS], F32)
    inp("ind_r", [1, NPAD], F32)
    for name, arr_shape, np_dtype in w_shapes:
        inp(name, list(arr_shape), F16 if np_dtype == NP16 else F32)

    out_em = nc.dram_tensor("out_em", [128, ET], F32, kind="ExternalOutput")

    with tile.TileContext(nc) as tc, ExitStack() as ctx:
        P = lambda name, bufs, **kw: ctx.enter_context(
            tc.tile_pool(name=name, bufs=bufs, **kw))
        const = P("const", 1)
        big = P("big", 1)
        dram = P("dram", 1, space="DRAM")
        selp = P("selp", 3)
        selTring = P("selTring", 3)
        gath = P("gath", 6)
        hring = P("hring", 6)
        hem = P("hem", 8)
        tring = P("tring", 4)
        sgring = P("sgring", 3)
        rows = P("rows", 2)
        aggring = P("aggring", 2)
        pqring = P("pqring", 2)
        ps_r = P("ps_r", 2, space="PSUM")
        ps_le = P("ps_le", 2, space="PSUM")
        ps_g = P("ps_g", 1, space="PSUM")
        ps_m = P("ps_m", 1, space="PSUM")
        ps_t = P("ps_t", 2, space="PSUM")

        # ---- load constants / weights ----
        W = {}
        for name, arr_shape, np_dtype in w_shapes:
            t = const.tile(list(arr_shape), F16 if np_dtype == NP16 else F32,
                           name=f"w_{name}")
            nc.sync.dma_start(t[:], din[name][:])
            W[name] = t
        gq_idx = const.tile([128, Epad // 16], I16, name="gq_idx_s")
        nc.sync.dma_start(gq_idx[:], din["gq_idx"][:])
        cnt = const.tile([128, BLOCKS], F32, name="cnt_s")
        nc.sync.dma_start(cnt[:], din["cnt_nm"][:])
        ind = const.tile([1, NPAD], F32, name="ind_s")
        nc.sync.dma_start(ind[:], din["ind_r"][:])
        ea_em = const.tile([128, ET], F32, name="ea_em_s")
        nc.sync.dma_start(ea_em[:], din["ea_em"][:])
        dm_em = const.tile([128, ET], F16, name="dm_em_s")
        nc.sync.dma_start(dm_em[:], din["dm_em"][:])
        dmc_em = const.tile([128, ET], F16, name="dmc_em_s")
        nc.sync.dma_start(dmc_em[:], din["dmc_em"][:])

        zeros = const.tile([128, 512], F16, name="zeros")
        nc.vector.memset(zeros[:], 0.0)
        invc = const.tile([128, BLOCKS], F32, name="invc")
        nc.vector.tensor_scalar_max(invc[:], cnt[:], 1.0)
        nc.vector.reciprocal(invc[:], invc[:])

        # ---- persistent big tensors ----
        le = big.tile([128, Epad], F16, name="le")       # edge latent (feature-major)
        ln = big.tile([128, NPAD], F32, name="ln")       # own-node latent
        sst = big.tile([128, NPAD], F32, name="sst")     # scaled segsum(h)^T
        p_own = big.tile([128, NPAD], F16, name="p_own")  # [slot, f] per block
        dec_em = big.tile([128, ET], F32, name="dec_em")
        cmb = big.tile([128, ET], F32, name="cmb")

        if kb_no_sel or kb_blocks < BLOCKS:
            nc.vector.memset(sst[:], 0.0)
        q_own = [dram.tile([NPAD, 128], F16, name=f"q_own_{s}") for s in range(S)]
        q_full = [dram.tile([TROWS, 128], F16, name=f"q_full_{s}",
                            addr_space="Shared") for s in range(S)]

        def mlp_rows(src_row, nrows, hidden_W0, b0, W1h, b1, dst, j0=0):
            """dst[:, 512j:...] = W1h.T @ relu(W0 (x) row_j + b0) + b1."""
            for j in range(j0, nrows):
                r = rows.tile([1, 512], hidden_W0.dtype, tag="rowin")
                nc.sync.dma_start(r[:], src_row[:, 512 * j:512 * (j + 1)])
                ps = ps_r.tile([128, 512], F32, tag="r")
                nc.tensor.matmul(ps[:], hidden_W0[:], r[:],
                                 start=True, stop=True)
                h0 = hring.tile([128, 512], F16, tag="h")
                nc.scalar.activation(h0[:], ps[:], AF.Relu, bias=b0[:])
                ps2 = ps_le.tile([128, 512], F32, tag="le")
                nc.tensor.matmul(ps2[:], W1h[:], h0[:], start=True, stop=True)
                nc.scalar.activation(dst[:, 512 * j:512 * (j + 1)], ps2[:],
                                     AF.Identity, bias=b1[:])

        # ---- node encoder (edge encoder emitted after AllGather 0) ----
        mlp_rows(din["x_row"], NPAD // 512, W["encn_W0"], W["encn_b0"],
                 W["encn_W1h"], W["encn_b1"], ln)

        def pq_block(s, j):
            l16 = hem.tile([128, 128], F16, tag="hem")
            nc.scalar.activation(l16[:], ln[:, 128 * j:128 * (j + 1)], AF.Copy)
            ps = ps_t.tile([128, 256], F32, tag="tp")
            nc.tensor.matmul(ps[:], l16[:], W[f"eW0ab_{s}"][:],
                             start=True, stop=True)
            nc.vector.tensor_tensor(
                p_own[:, 128 * j:128 * (j + 1)], ps[:, 0:128],
                W[f"eb0bc_{s}"][:], op=OP.add)
            t = pqring.tile([128, 128], F16, tag="pqe")
            nc.scalar.activation(t[:], ps[:, 128:256], AF.Copy)
            nc.sync.dma_start(q_own[s][128 * j:128 * (j + 1), :], t[:])

        def pq_cc(s):
            if kb_no_cc:
                nc.sync.dma_start(q_full[s][0:NPAD, :], q_own[s][:])
            else:
                nc.gpsimd.collective_compute(
                    "AllGather", OP.bypass,
                    replica_groups=[list(range(NCORES))],
                    ins=[q_own[s].opt()],
                    outs=[q_full[s].opt()],
                )

        def node_chunk(s, j):
            """agg + node MLP for node column chunk j (512 cols)."""
            o = 512 * j
            a_ps = ps_t.tile([128, 512], F32, tag="tp")
            nc.tensor.matmul(a_ps[:], W[f"eW1f_{s}"][:], sst[:, o:o + 512],
                             start=True, stop=False)
            nc.tensor.matmul(a_ps[:], W[f"eb1row_{s}"][:],
                             ind[:, o:o + 512], start=False, stop=True)
            agg = aggring.tile([128, 512], F16, tag="agg16")
            nc.scalar.activation(agg[:], a_ps[:], AF.Copy)
            p_ps = ps_r.tile([128, 512], F32, tag="r")
            nc.tensor.matmul(p_ps[:], W[f"nW0a_{s}"][:], ln[:, o:o + 512],
                             start=True, stop=False)
            nc.tensor.matmul(p_ps[:], W[f"nW0bh_{s}"][:], agg[:],
                             start=False, stop=True)
            hn = hring.tile([128, 512], F16, tag="h")
            nc.scalar.activation(hn[:], p_ps[:], AF.Relu,
                                 bias=W[f"nb0_{s}"][:])
            l_ps = ps_le.tile([128, 512], F32, tag="le")
            nc.tensor.matmul(l_ps[:], W[f"nW1h_{s}"][:], hn[:],
                             start=True, stop=True)
            nc.scalar.activation(ln[:, o:o + 512], l_ps[:],
                                 AF.Identity, bias=W[f"nb1_{s}"][:])

        for j in range(BLOCKS):
            pq_block(0, j)
        pq_cc(0)

        dec_done = 0

        def dec_chunk(ci):
            off = 512 * ci
            ps = ps_r.tile([128, 512], F32, tag="r")
            nc.tensor.matmul(ps[:], W["dec_W0h"][:], le[:, off:off + 512],
                             start=True, stop=True)
            hd = hring.tile([128, 512], F16, tag="h")
            nc.scalar.activation(hd[:], ps[:], AF.Relu, bias=W["dec_b0"][:])
            d_ps = ps_m.tile([128, 4], F32, tag="m")
            for u in range(4):
                nc.tensor.matmul(d_ps[:, u:u + 1], hd[:, 128 * u:128 * (u + 1)],
                                 W["dec_W1h"][:], start=True, stop=True)
            nc.vector.tensor_tensor(dec_em[:, 4 * ci:4 * ci + 4], d_ps[:],
                                    W["dec_b1"][:].to_broadcast([128, 4]),
                                    op=OP.add)

        def dec_upto(c_end):
            nonlocal dec_done
            while dec_done < c_end:
                dec_chunk(dec_done)
                dec_done += 1

        enc_done = 0

        def enc_upto(c_end):
            nonlocal enc_done
            if c_end > enc_done:
                mlp_rows(din["ea_row"], c_end, W["ence_W0"], W["ence_b0"],
                         W["ence_W1h"], W["ence_b1"], le, j0=enc_done)
                enc_done = c_end

        g_ctr = 0
        for s in range(S):
            do_agg = s < S - 1
            eW0c = W[f"eW0c_{s}"]
            eW1h = W[f"eW1h_{s}"]
            eb1 = W[f"eb1_{s}"]
            for b in range(kb_blocks):
                if s == 0:
                    enc_upto(min(NCH, ((b + 3) * EB + 511) // 512))
                selT_b = selTring.tile([128, EB], F8, tag="selT")
                nc.sync.dma_start(selT_b[:], din["selT"][:, b * EB:(b + 1) * EB])
                pob = p_own[:, 128 * b:128 * (b + 1)]
                if do_agg and not kb_no_sel:
                    g_ps = ps_g.tile([128, 128], F32, tag="g")
                for (gt0, gnt) in groups:
                    # gather this group's Q rows
                    i0 = b * EB + gt0 * 128
                    ni = gnt * 128
                    gq_t = gath.tile([128, 1, GT * 128], F16, tag="gq")
                    if kb_no_gather:
                        nc.vector.memset(gq_t[:], 0.0)
                    else:
                        nc.gpsimd.dma_gather(
                            gq_t[:, :, :ni], q_full[s][:, :],
                            gq_idx[:, i0 // 16:(i0 + ni) // 16],
                            num_idxs=ni, num_idxs_reg=ni,
                            elem_size=128, elem_step=128, transpose=True,
                            queue_num=g_ctr % 4)
                        g_ctr += 1
                    if do_agg and not kb_no_sel:
                        sel_t = selp.tile([128, GT, 128], F8, tag="sel")
                        nc.sync.dma_start(
                            sel_t[:, :gnt, :],
                            din["sel"][i0:i0 + ni, :].rearrange(
                                "(t p) s -> p t s", p=128))
                    # chunks of <=512 within the group
                    co = 0
                    while co < ni:
                        cw = min(512, ni - co)
                        goff = i0 + co            # global edge-slot offset
                        boff = gt0 * 128 + co     # offset within the block
                        ps = ps_r.tile([128, 512], F32, tag="r")
                        nc.tensor.matmul(ps[:, :cw], pob,
                                         selT_b[:, boff:boff + cw],
                                         start=True, stop=False)
                        nc.tensor.matmul(ps[:, :cw], eW0c[:],
                                         le[:, goff:goff + cw],
                                         start=False, stop=True)
                        t2 = tring.tile([128, 512], F16, tag="t")
                        nc.vector.tensor_tensor(t2[:, :cw],
                                                gq_t[:, 0, co:co + cw],
                                                ps[:, :cw], op=OP.add)
                        h = hring.tile([128, 512], F16, tag="h")
                        nc.vector.tensor_tensor(h[:, :cw], t2[:, :cw],
                                                zeros[:, :cw], op=OP.max)
                        ps2 = ps_le.tile([128, 512], F32, tag="le")
                        nc.tensor.matmul(ps2[:, :cw], eW1h[:], h[:, :cw],
                                         start=True, stop=True)
                        nc.scalar.activation(le[:, goff:goff + cw], ps2[:, :cw],
                                             AF.Identity, bias=eb1[:])
                        if do_agg and not kb_no_sel:
                            for u in range(cw // 128):
                                tt = gt0 + (co // 128) + u
                                he = hem.tile([128, 128], F16, tag="hem")
                                if kb_no_tp:
                                    nc.vector.tensor_copy(
                                        he[:], h[:, 128 * u:128 * (u + 1)])
                                else:
                                    ht_ps = ps_t.tile([128, 128], F16, tag="tp")
                                    nc.tensor.transpose(
                                        ht_ps[:], h[:, 128 * u:128 * (u + 1)],
                                        W["ident16"][:])
                                    if tt % 2 == 0:
                                        nc.scalar.activation(he[:], ht_ps[:],
                                                             AF.Copy)
                                    else:
                                        nc.vector.tensor_tensor(
                                            he[:], ht_ps[:], zeros[:, 0:128],
                                            op=OP.add)
                                nc.tensor.matmul(
                                    g_ps[:], sel_t[:, (co // 128) + u, :], he[:],
                                    start=(tt == 0), stop=(tt == Tb - 1),
                                    skip_group_check=True)
                        co += cw
                if s == S - 1 and not kb_no_dec:
                    dec_upto(((b + 1) * EB) // 512)
                if do_agg and not kb_no_sel:
                    sg = sgring.tile([128, 128], F32, tag="sg")
                    nc.vector.tensor_tensor(sg[:], g_ps[:],
                                            invc[:, b:b + 1].to_broadcast(
                                                [128, 128]), op=OP.mult)
                    ps_tr = ps_m.tile([128, 128], F32, tag="m")
                    nc.tensor.transpose(ps_tr[:], sg[:], W["ident"][:])
                    nc.vector.tensor_tensor(sst[:, 128 * b:128 * (b + 1)],
                                            ps_tr[:], zeros[:, 0:128], op=OP.add)
                    if b % 4 == 3 and kb_blocks == BLOCKS:
                        node_chunk(s, b // 4)
                        for jb in range(4 * (b // 4), 4 * (b // 4) + 4):
                            pq_block(s + 1, jb)

            if do_agg:
                pq_cc(s + 1)

        # ---- decoder leftovers (most chunks interleaved into step 2) ----
        if not kb_no_dec:
            dec_upto(NCH)

        # ---- final combine: out = dm*0.5*sqrt(ea) + dmc*dec ----
        nc.scalar.sqrt(cmb[:], ea_em[:])
        nc.vector.scalar_tensor_tensor(cmb[:], dm_em[:], 0.5, cmb[:],
                                       op0=OP.mult, op1=OP.mult)
        nc.vector.tensor_tensor(dec_em[:], dmc_em[:], dec_em[:], op=OP.mult)
        nc.vector.tensor_tensor(cmb[:], cmb[:], dec_em[:], op=OP.add)
        nc.sync.dma_start(out_em[:], cmb[:])

    nc.compile()


# ----------------------------------------------------------------------------
# Entry point
# ----------------------------------------------------------------------------

def _get_program(Tb, w_shapes):
    key = Tb
    if key not in _CACHE:
        import time
        t0 = time.time()
        nc = bacc.Bacc("TRN2", target_bir_lowering=False, debug=False,
                       num_devices=NCORES, num_swdge_queues=4)
        _build(nc, Tb, w_shapes)
        if os.environ.get("KERNEL_VERBOSE"):
            print(f"[kernel] build+schedule+compile: {time.time()-t0:.1f}s",
                  flush=True)
        _CACHE[key] = nc
    return _CACHE[key]


def kernel(**inputs):
    per_core, Tb = _prep(inputs["x"], inputs["edge_attr"], inputs["edge_index"])
    w = _weights_inputs(inputs)
    w_shapes = [(k, v.shape, v.dtype.type) for k, v in w.items()]
    nc = _get_program(Tb, w_shapes)

    in_maps = []
    for k in range(NCORES):
        m = dict(w)
        pc = per_core[k]
        for key in ("ea_row", "x_row", "ea_em", "dm_em", "dmc_em", "gq_idx",
                    "sel", "selT", "cnt_nm", "ind_r"):
            m[key] = pc[key]
        in_maps.append(m)

    trace = bool(int(os.environ.get("KERNEL_TRACE", "0")))
    import time as _time
    _t0 = _time.time()
    if not trace:
        # Warm-up execution: the very first run of a freshly compiled NEFF
        # has been observed to return corrupted results; discard it.
        run_bass_kernel_spmd(nc, in_maps, core_ids=list(range(NCORES)),
                             trace=False)
    res = run_bass_kernel_spmd(
        nc, in_maps, core_ids=list(range(NCORES)), trace=trace,
        tmpdir=os.environ.get("KERNEL_TRACE_DIR") or None)
    if os.environ.get("KERNEL_VERBOSE"):
        print(f"[kernel] exec phase: {_time.time()-_t0:.1f}s", flush=True)
    if trace:
        print(f"HW exec time: {res.exec_time_ns} ns")
        if res.instructions_and_trace:
            print("trace:", res.instructions_and_trace[1])

    out = np.zeros((E, 1), dtype=np.float32)
    ET = (BLOCKS * Tb * 128) // 128
    for k in range(NCORES):
        o = res.results[k]["out_em"]           # [128, ET]
        flat = o.T.reshape(-1)                 # slot order
        orig = per_core[k]["orig"]
        valid = orig >= 0
        out[orig[valid], 0] = flat[valid]
    return out

